# revision 2
# baseline (speedup 1.0000x reference)
"""Trainium2 Bass kernel v2 for nn_Attention (qkv+BN1 -> biased softmax attn -> gelu -> proj; BN2 on host).

Data-parallel over batch B=128 across 8 cores (16 batches = 8 "pairs" of 640 rows each).

Key structure (per core):
  - Host sends xT [256,5120] bf16 (moving operand for q/k/v matmuls) and
    x_rows packed [128, 40*256] bf16 (stationary for the x Gram matrix).
  - BN1 stats via Gram: var(qkv_h) = W_h (Sigma_x) W_h^T - mean1^2, with
    Sigma_x = sum_r x x^T allreduced EARLY (overlaps q/k/v matmuls).
  - Attention S[m-part, n-free] per (head, pair); the two 64-row mc2 chunks of
    a batch-pair share one PSUM tile (5 exps per (h,pair) instead of 6).
  - et = exp(scale*S) (Act) then *= exp(bias) gathered on host (DVE/Pool).
  - AV flipped: out[n, d] = sum_m et[m,n] v[m,d] with v stored [m, 8*(128+1)]
    (interleaved ones columns) so out col 128 = softmax denominator Z[n].
  - divide+BN1-v-beta: g_pre = (av * (1/Z)[n-part]) + beta_bc (Pool stt);
    alpha_v folded into the v PSUM->SBUF copy; then DMA-transpose to g_pair
    [d-part, row-free]; gelu per pair; proj -> y [c-part, row] fp32.
    BN2 (training-mode batchnorm, final linear op) done on HOST.
"""
import os
import numpy as np
import ml_dtypes

import concourse.bass as bass
import concourse.tile as tile
from concourse import bacc, mybir
from concourse.bass_utils import run_bass_kernel_spmd

NCORES = 8
TRACE_SIM = os.environ.get("KERN_TRACE_SIM", "") == "1"
DBG_STOP = os.environ.get("KERN_STOP", "")
B, N, C = 128, 320, 256
NH, DK, DV = 8, 32, 128
BL = B // NCORES             # 16 batches/core
R = BL * N                   # 5120 rows/core
NPAIR = BL // 2              # 8 pairs
PW = 2 * N                   # 640 rows/pair
NT = B * N                   # 40960 global rows
EPS = 1e-5
SCALE = DK ** -0.5
FP = mybir.dt.float32
BF = mybir.dt.bfloat16
AF = mybir.ActivationFunctionType
OP = mybir.AluOpType

S_BUFS = int(os.environ.get("K2_S_BUFS", "3"))
AV_BUFS = int(os.environ.get("K2_AV_BUFS", "3"))
VMM_BUFS = int(os.environ.get("K2_VMM_BUFS", "1"))
PROJ_BUFS = int(os.environ.get("K2_PROJ_BUFS", "1"))
ET_BUFS = int(os.environ.get("K2_ET_BUFS", "8"))
GP_BUFS = int(os.environ.get("K2_GP_BUFS", "3"))
EBM_POOL = int(os.environ.get("K2_EBM_POOL", "5"))  # of 5 eb-mults on Pool


def build_program():
    nc = bacc.Bacc("TRN2", target_bir_lowering=False, debug=False,
                   enable_asserts=False, num_devices=NCORES)
    xr_d = nc.dram_tensor("xr", [128, 40 * C], BF, kind="ExternalInput").ap()
    xt_d = nc.dram_tensor("xt", [C, R], BF, kind="ExternalInput").ap()
    wqk_d = nc.dram_tensor("wqk", [C, 2 * NH * DK], BF, kind="ExternalInput").ap()
    wv_d = nc.dram_tensor("wv", [C, NH * DV], BF, kind="ExternalInput").ap()
    wp_d = nc.dram_tensor("wp", [DV, NH * C], BF, kind="ExternalInput").ap()
    eb_d = nc.dram_tensor("eb", [128, NH * 3 * N], BF, kind="ExternalInput").ap()
    g1c_d = nc.dram_tensor("g1c", [128, 12], FP, kind="ExternalInput").ap()
    b1c_d = nc.dram_tensor("b1c", [128, 12], FP, kind="ExternalInput").ap()
    id_d = nc.dram_tensor("ident", [128, 128], FP, kind="ExternalInput").ap()
    y_d = nc.dram_tensor("y", [C, R], FP, kind="ExternalOutput").ap()

    with tile.TileContext(nc, trace_sim=TRACE_SIM) as tc:
        with tc.tile_pool(name="const", bufs=1) as constp, \
             tc.tile_pool(name="pers", bufs=1) as persp, \
             tc.tile_pool(name="dram", bufs=1, space="DRAM") as dramp:

            # ---------------- constants / inputs ----------------
            wqk_sb = [constp.tile([128, 2 * NH * DK], BF, name=f"wqk{cc}")
                      for cc in range(2)]
            wv_sb = [constp.tile([128, NH * DV], BF, name=f"wv{cc}")
                     for cc in range(2)]
            wp_sb = constp.tile([128, NH * C], BF)
            eb_sb = constp.tile([128, NH * 3 * N], BF)
            g1c_sb = constp.tile([128, 12], FP)
            b1c_sb = constp.tile([128, 12], FP)
            id_sb = constp.tile([128, 128], FP)
            xt_sb = [persp.tile([128, R], BF, name=f"xt{cc}") for cc in range(2)]
            ones_bf = constp.tile([128, 1], BF)
            nc.vector.memset(ones_bf[:], 1.0)
            onesrow_bf = constp.tile([1, 128], BF)
            nc.vector.memset(onesrow_bf[:], 1.0)

            # persistent
            qk_sb = [persp.tile([128, R], BF, name=f"qk{hc}") for hc in range(4)]
            alpha1 = persp.tile([128, 12], FP)
            beta1 = persp.tile([128, 12], FP)
            stats = persp.tile([128, 768], FP)
            statsg = persp.tile([128, 768], FP)
            gram_bf = persp.tile([128, 512], BF)
            meanx_bf = persp.tile([128, 2], BF)

            # W blocks for stats matmuls: (tiles, col0, width, flat-offset)
            wblocks = [(wqk_sb, 0, 512, 0),
                       (wv_sb, 0, 512, 512),
                       (wv_sb, 512, 512, 1024)]

            # ---------------- Phase A ----------------
            with tc.tile_pool(name="pA", bufs=2, space="PSUM") as pA, \
                 tc.tile_pool(name="sA", bufs=1) as sA:
                mean1s = sA.tile([128, 12], FP, tag="m1")
                vars_ = sA.tile([128, 12], FP, tag="vf")
                tmps = sA.tile([128, 12], FP, tag="tf")
                rstds = sA.tile([128, 12], FP, tag="rf")
                p_sb = [sA.tile([128, 1536], BF, tag=f"pp{c2}", name=f"p_sb{c2}")
                        for c2 in range(2)]
                with tc.tile_pool(name="xrp", bufs=1) as xrp, \
                     tc.tile_pool(name="pG", bufs=1, space="PSUM") as pG:
                    xr_sb = xrp.tile([128, 40 * C], BF)
                    for xh in range(4):
                        nc.sync.dma_start(
                            xr_sb[:, xh * 10 * C:(xh + 1) * 10 * C],
                            xr_d[:, xh * 10 * C:(xh + 1) * 10 * C])
                    nc.vector.memset(stats[:, 512:768], 0.0)
                    for cc in range(2):
                        nc.sync.dma_start(xt_sb[cc][:],
                                          xt_d[cc * 128:(cc + 1) * 128, :])
                        nc.sync.dma_start(wqk_sb[cc][:],
                                          wqk_d[cc * 128:(cc + 1) * 128, :])
                        nc.sync.dma_start(wv_sb[cc][:],
                                          wv_d[cc * 128:(cc + 1) * 128, :])
                    nc.scalar.dma_start(wp_sb[:], wp_d[:])
                    nc.scalar.dma_start(eb_sb[:], eb_d[:])
                    nc.scalar.dma_start(g1c_sb[:], g1c_d[:])
                    nc.scalar.dma_start(b1c_sb[:], b1c_d[:])
                    nc.scalar.dma_start(id_sb[:], id_d[:])
                    gps = [pG.tile([128, C], FP, tag=f"g{cc}", name=f"gps{cc}")
                           for cc in range(2)]
                    csp = pG.tile([1, C], FP, tag="cs", name="csp")
                    for rb in range(40):
                        nc.tensor.matmul(
                            gps[0][:],
                            xr_sb[:, rb * C: rb * C + 128],
                            xr_sb[:, rb * C:(rb + 1) * C],
                            start=(rb == 0), stop=(rb == 39))
                        nc.tensor.matmul(
                            gps[1][:, 128:256],
                            xr_sb[:, rb * C + 128: rb * C + 256],
                            xr_sb[:, rb * C + 128:(rb + 1) * C],
                            start=(rb == 0), stop=(rb == 39))
                        nc.tensor.matmul(
                            csp[:], ones_bf[:], xr_sb[:, rb * C:(rb + 1) * C],
                            start=(rb == 0), stop=(rb == 39))
                    nc.vector.tensor_copy(stats[:, 0:256], gps[0][:])
                    nc.vector.memset(stats[:, 256:384], 0.0)
                    nc.vector.tensor_copy(stats[:, 384:512], gps[1][:, 128:256])
                    nc.vector.tensor_copy(stats[0:1, 512:768], csp[:])
                bi = dramp.tile([128, 768], FP, tag="bi")
                bo = dramp.tile([128, 768], FP, tag="bo")
                nc.gpsimd.dma_start(bi[:], stats[:])
                nc.gpsimd.collective_compute(
                    "AllReduce", OP.add,
                    replica_groups=[list(range(NCORES))],
                    ins=[bi.opt()], outs=[bo.opt()])
                nc.gpsimd.dma_start(statsg[:], bo[:])

                # q/k matmuls (independent of stats; overlap the collective)
                for hc in range(4):
                    for rb in range(10):
                        pq = pA.tile([128, 512], FP, tag="pq")
                        for cc in range(2):
                            nc.tensor.matmul(
                                pq[:], wqk_sb[cc][:, hc * 128:(hc + 1) * 128],
                                xt_sb[cc][:, rb * 512:(rb + 1) * 512],
                                start=(cc == 0), stop=(cc == 1))
                        nc.scalar.copy(qk_sb[hc][:, rb * 512:(rb + 1) * 512],
                                       pq[:])

                # ---- BN1 stats math (gated on allreduce) ----
                with tc.tile_pool(name="pS1", bufs=1, space="PSUM") as pS1:
                    nc.gpsimd.tensor_copy(gram_bf[:, 0:128], statsg[:, 0:128])
                    nc.gpsimd.tensor_scalar(gram_bf[:, 128:256],
                                            statsg[:, 128:256], 2.0, None, OP.mult)
                    nc.gpsimd.tensor_copy(gram_bf[:, 384:512], statsg[:, 384:512])
                    for cc in range(2):
                        tpm = pS1.tile([128, 2], FP, tag="tp", bufs=1)
                        nc.tensor.transpose(
                            tpm[:, 0:1],
                            statsg[0:1, 512 + cc * 128:512 + (cc + 1) * 128],
                            id_sb[0:1, 0:1])
                        nc.vector.tensor_scalar(meanx_bf[:, cc:cc + 1], tpm[:, 0:1],
                                                1.0 / NT, None, OP.mult)
                    # mean1 [128,12] direct: out[:, hc] = sum_c W[c, hc-dims] mean_x[c]
                    m1p = pS1.tile([128, 12], FP, tag="m1p")
                    wcols = [(wqk_sb, 0), (wqk_sb, 128), (wqk_sb, 256), (wqk_sb, 384),
                             (wv_sb, 0), (wv_sb, 128), (wv_sb, 256), (wv_sb, 384),
                             (wv_sb, 512), (wv_sb, 640), (wv_sb, 768), (wv_sb, 896)]
                    for hc, (wt, c0) in enumerate(wcols):
                        for cc in range(2):
                            nc.tensor.matmul(m1p[:, hc:hc + 1],
                                             wt[cc][:, c0:c0 + 128],
                                             meanx_bf[:, cc:cc + 1],
                                             start=(cc == 0), stop=(cc == 1))
                    nc.vector.tensor_copy(mean1s[:], m1p[:])
                    # E2 decomposition using symmetric Gram:
                    # P0 = W0 .* (G00 W0); P1 = W1 .* (G11 W1 + 2 G01^T W0)
                    # where G01^T W0 is computed as M01[c2-chunk1, h] via
                    # stationary G01 [c1-part, c2-cols].
                    for (wt, c0, w, f0) in wblocks:
                        mm = pS1.tile([128, 512], FP, tag="mm")
                        nc.tensor.matmul(mm[:], gram_bf[:, 0:128],
                                         wt[0][:, c0:c0 + w])
                        nc.vector.tensor_tensor(p_sb[0][:, f0:f0 + w], mm[:],
                                                wt[0][:, c0:c0 + w], OP.mult)
                        mm2 = pS1.tile([128, 512], FP, tag="mm")
                        nc.tensor.matmul(mm2[:], gram_bf[:, 384:512],
                                         wt[1][:, c0:c0 + w], start=True, stop=False)
                        nc.tensor.matmul(mm2[:], gram_bf[:, 128:256],
                                         wt[0][:, c0:c0 + w], start=False, stop=True)
                        nc.vector.tensor_tensor(p_sb[1][:, f0:f0 + w], mm2[:],
                                                wt[1][:, c0:c0 + w], OP.mult)
                    # E2 [128,12] direct: out[:, hc] = sum_c2 P[c2, hc-dims]
                    e2p = pS1.tile([128, 12], FP, tag="e2p")
                    for hc in range(12):
                        for c2 in range(2):
                            nc.tensor.matmul(e2p[:, hc:hc + 1],
                                             p_sb[c2][:, hc * 128:(hc + 1) * 128],
                                             ones_bf[:],
                                             start=(c2 == 0), stop=(c2 == 1))
                    # var = E2/NT - mean1^2 + eps ; rstd = exp(-0.5 ln(var))
                    nc.vector.tensor_tensor(tmps[:], mean1s[:], mean1s[:], OP.mult)
                    nc.vector.tensor_scalar(tmps[:], tmps[:], -1.0, EPS, OP.mult,
                                            OP.add)
                    nc.vector.scalar_tensor_tensor(vars_[:], e2p[:], 1.0 / NT,
                                                   tmps[:], OP.mult, OP.add)
                    nc.scalar.activation(tmps[:], vars_[:], AF.Ln)
                    nc.scalar.activation(rstds[:], tmps[:], AF.Exp, scale=-0.5)
                    nc.vector.tensor_tensor(alpha1[:], g1c_sb[:], rstds[:], OP.mult)
                    nc.vector.tensor_tensor(tmps[:], mean1s[:], alpha1[:], OP.mult)
                    nc.vector.tensor_tensor(beta1[:], b1c_sb[:], tmps[:], OP.subtract)

            if DBG_STOP == "A":
                for hc in range(4):
                    nc.vector.tensor_scalar(
                        qk_sb[hc][:], qk_sb[hc][:],
                        alpha1[:, hc:hc + 1], beta1[:, hc:hc + 1], OP.mult, OP.add)
                with tc.tile_pool(name="dbg", bufs=2) as dbgp:
                    for i, hc in enumerate((0, 2)):
                        for rb in range(10):
                            dq = dbgp.tile([128, 512], FP, tag="dq", name="dq")
                            nc.vector.tensor_copy(
                                dq[:], qk_sb[hc][:, rb * 512:(rb + 1) * 512])
                            nc.sync.dma_start(
                                y_d[i * 128:(i + 1) * 128, rb * 512:(rb + 1) * 512],
                                dq[:])
            else:
                # ---------- attention + output, pair-streamed ----------
                with tc.tile_pool(name="pSm", bufs=S_BUFS, space="PSUM") as pSm, \
                     tc.tile_pool(name="pAv", bufs=AV_BUFS, space="PSUM") as pAv, \
                     tc.tile_pool(name="pVm", bufs=VMM_BUFS, space="PSUM") as pVm, \
                     tc.tile_pool(name="pPj", bufs=PROJ_BUFS, space="PSUM") as pPj, \
                     tc.tile_pool(name="etp", bufs=ET_BUFS) as etp, \
                     tc.tile_pool(name="vtp", bufs=3) as vtp, \
                     tc.tile_pool(name="gpp", bufs=10) as gpp, \
                     tc.tile_pool(name="gpr", bufs=GP_BUFS) as gpr, \
                     tc.tile_pool(name="ysp", bufs=2) as ysp, \
                     tc.tile_pool(name="rrp", bufs=6) as rrp:
                    # v m-chunk windows in a pair: (name, row0-in-pair, width)
                    VCH = [("c0", 0, 128), ("c1", 128, 128),
                           ("c2", N, 128), ("c3", N + 128, 128)]
                    g_pairs = {}
                    for pair in range(NPAIR):
                        r0 = pair * PW
                        for hc in range(4):
                            nc.gpsimd.tensor_scalar(
                                qk_sb[hc][:, r0:r0 + PW], qk_sb[hc][:, r0:r0 + PW],
                                alpha1[:, hc:hc + 1], beta1[:, hc:hc + 1],
                                OP.mult, OP.add)
                        # v matmuls -> vT chunk tiles [m, 8*(128+1)] with alpha fold
                        vts = {}
                        for (cn, rr, wdt) in VCH:
                            vt = vtp.tile([128, NH * (DV + 1)], BF, tag=f"vt{cn}",
                                          name=f"vt{cn}")
                            vts[cn] = vt
                            vt3 = vt.rearrange("p (h x) -> p h x", x=DV + 1)
                            for dh in range(2):
                                vp = pVm.tile([128, 512], FP, tag="vp")
                                for cc in range(2):
                                    nc.tensor.matmul(
                                        vp[:], xt_sb[cc][:, r0 + rr:r0 + rr + wdt],
                                        wv_sb[cc][:, dh * 512:(dh + 1) * 512],
                                        start=(cc == 0), stop=(cc == 1))
                                nc.vector.tensor_copy(
                                    vt3[:, dh * 4:dh * 4 + 4, 0:128], vp[:])
                            nc.gpsimd.memset(vt[:, 128::129], 1.0)
                        vt = vtp.tile([128, NH * (DV + 1)], BF, tag="vt4", name="vt4")
                        vts["c4"] = vt
                        vt3 = vt.rearrange("p (h x) -> p h x", x=DV + 1)
                        for dh in range(2):
                            vp = pVm.tile([128, 512], FP, tag="vp")
                            for (bb, orow) in ((0, 0), (1, 64)):
                                for cc in range(2):
                                    nc.tensor.matmul(
                                        vp[orow:orow + 64, :],
                                        xt_sb[cc][:, r0 + bb * N + 256:
                                                 r0 + bb * N + 320],
                                        wv_sb[cc][:, dh * 512:(dh + 1) * 512],
                                        start=(cc == 0), stop=(cc == 1))
                            nc.vector.tensor_copy(
                                vt3[:, dh * 4:dh * 4 + 4, 0:128], vp[:])
                        nc.gpsimd.memset(vt[:, 128::129], 1.0)

                        g_pair = gpr.tile([128, NH * PW], BF, tag="gp", name="g_pair")
                        g_pairs[pair] = g_pair
                        for h in range(NH):
                            qc, qr = h // 4, 32 * (h % 4)
                            q0 = qk_sb[qc][qr:qr + 32, r0:r0 + N]
                            q1 = qk_sb[qc][qr:qr + 32, r0 + N:r0 + PW]
                            k0 = qk_sb[2 + qc][qr:qr + 32, r0:r0 + N]
                            k1 = qk_sb[2 + qc][qr:qr + 32, r0 + N:r0 + PW]
                            sdefs = [("s0", k0[:, 0:128], q0),
                                     ("s1", k0[:, 128:256], q0),
                                     ("s2", k1[:, 0:128], q1),
                                     ("s3", k1[:, 128:256], q1)]
                            ets = []
                            for (tg, kap, qap) in sdefs:
                                sp = pSm.tile([128, N], FP, tag="s", name="sp")
                                nc.tensor.matmul(sp[:], kap, qap,
                                                 tile_position=(qr, 0))
                                et = etp.tile([128, N], BF, tag=tg, name="et")
                                nc.scalar.activation(et[:], sp[:], AF.Exp, scale=SCALE)
                                ets.append(et)
                            sp = pSm.tile([128, N], FP, tag="s", name="sp4")
                            nc.tensor.matmul(sp[0:64, :], k0[:, 256:320], q0,
                                             tile_position=(qr, 0))
                            nc.tensor.matmul(sp[64:128, :], k1[:, 256:320], q1,
                                             tile_position=(qr, 64))
                            et4 = etp.tile([128, N], BF, tag="s4", name="et4")
                            nc.scalar.activation(et4[:], sp[:], AF.Exp, scale=SCALE)
                            ets.append(et4)
                            ebmap = [0, 1, 0, 1, 2]
                            for i, et in enumerate(ets):
                                ebs = eb_sb[:, (h * 3 + ebmap[i]) * N:
                                            (h * 3 + ebmap[i] + 1) * N]
                                if i < EBM_POOL:
                                    nc.gpsimd.tensor_tensor(et[:], et[:], ebs, OP.mult)
                                else:
                                    nc.vector.tensor_tensor(et[:], et[:], ebs, OP.mult)
                            avA = pAv.tile([128, 3 * 129], FP, tag="av", name="avA")
                            avB = pAv.tile([128, 3 * 129], FP, tag="av", name="avB")
                            regions = [(avA, 0, 0, 0), (avA, 129, 0, 1),
                                       (avB, 0, 1, 0), (avB, 129, 1, 1)]
                            vmap = [["c0", "c1", "c4"], ["c2", "c3", "c4"]]
                            emap = [[0, 1, 4], [2, 3, 4]]
                            for (dst, c0_, bb, ncx) in regions:
                                for mc in range(3):
                                    et = ets[emap[bb][mc]]
                                    vtt = vts[vmap[bb][mc]]
                                    if mc == 2:
                                        sr = bb * 64
                                        stp = et[sr:sr + 64, ncx * 128:ncx * 128 + 128]
                                        mvp = vtt[sr:sr + 64, h * 129:(h + 1) * 129]
                                    else:
                                        stp = et[:, ncx * 128:ncx * 128 + 128]
                                        mvp = vtt[:, h * 129:(h + 1) * 129]
                                    nc.tensor.matmul(dst[:, c0_:c0_ + 129], stp, mvp,
                                                     start=(mc == 0), stop=(mc == 2))
                            for bb in range(2):
                                orow = bb * 64
                                for mc in range(3):
                                    et = ets[emap[bb][mc]]
                                    vtt = vts[vmap[bb][mc]]
                                    if mc == 2:
                                        sr = bb * 64
                                        stp = et[sr:sr + 64, 256:320]
                                        mvp = vtt[sr:sr + 64, h * 129:(h + 1) * 129]
                                    else:
                                        stp = et[:, 256:320]
                                        mvp = vtt[:, h * 129:(h + 1) * 129]
                                    nc.tensor.matmul(
                                        avA[orow:orow + 64, 258:387], stp, mvp,
                                        start=(mc == 0), stop=(mc == 2))
                            rA = rrp.tile([128, 3], FP, tag="rA", name="rA")
                            rB = rrp.tile([128, 2], FP, tag="rB", name="rB")
                            nc.vector.reciprocal_approx_fast(rA[:], avA[:, 128::129])
                            nc.vector.reciprocal_approx_fast(rB[:],
                                                             avB[:, 128:300:129])
                            ddefs = [(avA, 0, rA, 0, 0), (avA, 129, rA, 1, 128),
                                     (avB, 0, rB, 0, N), (avB, 129, rB, 1, N + 128)]
                            for (src, c0_, rr_, ri, gc) in ddefs:
                                gp = gpp.tile([128, 128], BF, tag="gpre", name="gp")
                                nc.vector.tensor_scalar(
                                    gp[:], src[:, c0_:c0_ + 128], rr_[:, ri:ri + 1],
                                    None, OP.mult)
                                nc.sync.dma_start_transpose(
                                    g_pair[:, h * PW + gc:h * PW + gc + 128], gp[:])
                            gp = gpp.tile([128, 128], BF, tag="gpre", name="gp4")
                            nc.vector.tensor_scalar(
                                gp[:], avA[:, 258:386], rA[:, 2:3],
                                None, OP.mult)
                            nc.sync.dma_start_transpose(
                                g_pair[:, h * PW + 256:h * PW + 320], gp[0:64, :])
                            nc.sync.dma_start_transpose(
                                g_pair[:, h * PW + N + 256:h * PW + N + 320],
                                gp[64:128, :])
                        for h in range(NH):
                            nc.gpsimd.tensor_scalar(
                                g_pair[:, h * PW:(h + 1) * PW],
                                g_pair[:, h * PW:(h + 1) * PW],
                                alpha1[:, 4 + h:5 + h], beta1[:, 4 + h:5 + h],
                                OP.mult, OP.add)
                        flush = ((pair % 2 == 1 and pair < 6) or pair >= 6)
                        if flush:
                            plist = [pair] if pair >= 6 else [pair - 1, pair]
                            for p2 in plist:
                                nc.scalar.activation(g_pairs[p2][:], g_pairs[p2][:],
                                                     AF.Gelu)
                            for p2 in plist:
                                gp2 = g_pairs.pop(p2)
                                rr0 = p2 * PW
                                for cc in range(2):
                                    yst = ysp.tile([128, PW], FP, tag=f"y{cc}",
                                                   name="yst")
                                    for half in range(2):
                                        pp = pPj.tile([128, N], FP, tag="pp")
                                        for h in range(NH):
                                            nc.tensor.matmul(
                                                pp[:],
                                                wp_sb[:, h * C + cc * 128:
                                                      h * C + cc * 128 + 128],
                                                gp2[:, h * PW + half * N:
                                                    h * PW + (half + 1) * N],
                                                start=(h == 0), stop=(h == NH - 1))
                                        nc.vector.tensor_copy(
                                            yst[:, half * N:(half + 1) * N], pp[:])
                                    nc.sync.dma_start(
                                        y_d[cc * 128:(cc + 1) * 128, rr0:rr0 + PW],
                                        yst[:])

    nc.compile()
    return nc


_PROG = None


def _get_prog():
    global _PROG
    if _PROG is None:
        _PROG = build_program()
    return _PROG


def _host_prep(x, Wqkv, g1, b1, ab, Wproj, g2, b2, idxs):
    x = np.asarray(x, dtype=np.float32)
    Wqkv = np.asarray(Wqkv, dtype=np.float32)
    Wproj = np.asarray(Wproj, dtype=np.float32)
    g1 = np.asarray(g1, np.float32)
    b1 = np.asarray(b1, np.float32)
    idxs = np.asarray(idxs)
    qrows = np.concatenate([np.arange(h * 192, h * 192 + 32) for h in range(NH)])
    krows = np.concatenate([np.arange(h * 192 + 32, h * 192 + 64) for h in range(NH)])
    vrows = np.concatenate([np.arange(h * 192 + 64, h * 192 + 192) for h in range(NH)])
    wqk = np.ascontiguousarray(Wqkv[np.concatenate([qrows, krows]), :].T).astype(
        ml_dtypes.bfloat16)                                     # (256, 512)
    wv = np.ascontiguousarray(Wqkv[vrows, :].T).astype(ml_dtypes.bfloat16)
    wp = np.ascontiguousarray(
        Wproj.reshape(C, NH, DV).transpose(2, 1, 0).reshape(DV, NH * C)).astype(
        ml_dtypes.bfloat16)                                     # (128, 2048)
    perm = np.concatenate([qrows, krows, vrows])
    g1c = np.ascontiguousarray(g1[perm].reshape(12, 128).T)
    b1c = np.ascontiguousarray(b1[perm].reshape(12, 128).T)
    ebm = np.exp(np.asarray(ab, np.float32))[:, idxs]           # (8, 320, 320)
    ebp = np.zeros((128, NH * 3 * N), np.float32)
    for h in range(NH):
        ebp[:, (h * 3) * N:(h * 3 + 1) * N] = ebm[h, 0:128, :]
        ebp[:, (h * 3 + 1) * N:(h * 3 + 2) * N] = ebm[h, 128:256, :]
        ebp[0:64, (h * 3 + 2) * N:(h * 3 + 3) * N] = ebm[h, 256:320, :]
        ebp[64:128, (h * 3 + 2) * N:(h * 3 + 3) * N] = ebm[h, 256:320, :]
    common = {
        "wqk": wqk, "wv": wv, "wp": wp,
        "eb": ebp.astype(ml_dtypes.bfloat16),
        "g1c": g1c, "b1c": b1c,
        "ident": np.eye(128, dtype=np.float32),
    }
    in_maps = []
    for c in range(NCORES):
        xc = np.ascontiguousarray(x[c * BL:(c + 1) * BL].reshape(R, C))
        m = dict(common)
        m["xr"] = np.ascontiguousarray(
            xc.reshape(40, 128, C).transpose(1, 0, 2).reshape(128, 40 * C)).astype(
            ml_dtypes.bfloat16)
        m["xt"] = np.ascontiguousarray(xc.T).astype(ml_dtypes.bfloat16)
        in_maps.append(m)
    return in_maps


def _postprocess(res, g2, b2):
    ys = [np.asarray(res.results[c]["y"], np.float32) for c in range(NCORES)]
    z = np.concatenate(
        [y.T.reshape(BL, N, C) for y in ys], axis=0)            # (128, 320, 256)
    zf = z.reshape(-1, C).astype(np.float64)
    m = zf.mean(axis=0)
    v = zf.var(axis=0)
    out = (z - m.astype(np.float32)) * (1.0 / np.sqrt(v + EPS)).astype(np.float32) \
        * np.asarray(g2, np.float32) + np.asarray(b2, np.float32)
    return out.astype(np.float32)


def _run(in_maps, g2, b2, trace=False):
    nc = _get_prog()
    res = run_bass_kernel_spmd(nc, in_maps, core_ids=list(range(NCORES)),
                               trace=trace)
    return _postprocess(res, g2, b2), res


def kernel(**inputs):
    out, _ = _run(_host_prep(**inputs), inputs["g2"], inputs["b2"])
    return out


def run_raw(**inputs):
    """Return raw per-core y tensors (for debugging)."""
    nc = _get_prog()
    res = run_bass_kernel_spmd(nc, _host_prep(**inputs),
                               core_ids=list(range(NCORES)))
    return [np.asarray(res.results[c]["y"], np.float32) for c in range(NCORES)]


# revision 3
# speedup vs baseline: 1.0165x; 1.0165x over previous
"""Trainium2 Bass kernel v2 for nn_Attention (qkv+BN1 -> biased softmax attn -> gelu -> proj; BN2 on host).

Data-parallel over batch B=128 across 8 cores (16 batches = 8 "pairs" of 640 rows each).

Key structure (per core):
  - Host sends xT [256,5120] bf16 (moving operand for q/k/v matmuls) and
    x_rows packed [128, 40*256] bf16 (stationary for the x Gram matrix).
  - BN1 stats via Gram: var(qkv_h) = W_h (Sigma_x) W_h^T - mean1^2, with
    Sigma_x = sum_r x x^T allreduced EARLY (overlaps q/k/v matmuls).
  - Attention S[m-part, n-free] per (head, pair); the two 64-row mc2 chunks of
    a batch-pair share one PSUM tile (5 exps per (h,pair) instead of 6).
  - et = exp(scale*S) (Act) then *= exp(bias) gathered on host (DVE/Pool).
  - AV flipped: out[n, d] = sum_m et[m,n] v[m,d] with v stored [m, 8*(128+1)]
    (interleaved ones columns) so out col 128 = softmax denominator Z[n].
  - divide+BN1-v-beta: g_pre = (av * (1/Z)[n-part]) + beta_bc (Pool stt);
    alpha_v folded into the v PSUM->SBUF copy; then DMA-transpose to g_pair
    [d-part, row-free]; gelu per pair; proj -> y [c-part, row] fp32.
    BN2 (training-mode batchnorm, final linear op) done on HOST.
"""
import os
import numpy as np
import ml_dtypes

import concourse.bass as bass
import concourse.tile as tile
from concourse import bacc, mybir
from concourse.bass_utils import run_bass_kernel_spmd

NCORES = 8
TRACE_SIM = os.environ.get("KERN_TRACE_SIM", "") == "1"
DBG_STOP = os.environ.get("KERN_STOP", "")
B, N, C = 128, 320, 256
NH, DK, DV = 8, 32, 128
BL = B // NCORES             # 16 batches/core
R = BL * N                   # 5120 rows/core
NPAIR = BL // 2              # 8 pairs
PW = 2 * N                   # 640 rows/pair
NT = B * N                   # 40960 global rows
EPS = 1e-5
SCALE = DK ** -0.5
FP = mybir.dt.float32
BF = mybir.dt.bfloat16
AF = mybir.ActivationFunctionType
OP = mybir.AluOpType

S_BUFS = int(os.environ.get("K2_S_BUFS", "4"))
AV_BUFS = int(os.environ.get("K2_AV_BUFS", "2"))
VMM_BUFS = int(os.environ.get("K2_VMM_BUFS", "1"))
PROJ_BUFS = int(os.environ.get("K2_PROJ_BUFS", "1"))
ET_BUFS = int(os.environ.get("K2_ET_BUFS", "8"))
GP_BUFS = int(os.environ.get("K2_GP_BUFS", "3"))
EBM_POOL = int(os.environ.get("K2_EBM_POOL", "5"))  # of 5 eb-mults on Pool


def build_program():
    nc = bacc.Bacc("TRN2", target_bir_lowering=False, debug=False,
                   enable_asserts=False, num_devices=NCORES)
    xr_d = nc.dram_tensor("xr", [128, 40 * C], BF, kind="ExternalInput").ap()
    xt_d = nc.dram_tensor("xt", [C, R], BF, kind="ExternalInput").ap()
    wqk_d = nc.dram_tensor("wqk", [C, 2 * NH * DK], BF, kind="ExternalInput").ap()
    wv_d = nc.dram_tensor("wv", [C, NH * DV], BF, kind="ExternalInput").ap()
    wp_d = nc.dram_tensor("wp", [DV, NH * C], BF, kind="ExternalInput").ap()
    eb_d = nc.dram_tensor("eb", [128, NH * 3 * N], BF, kind="ExternalInput").ap()
    g1c_d = nc.dram_tensor("g1c", [128, 12], FP, kind="ExternalInput").ap()
    b1c_d = nc.dram_tensor("b1c", [128, 12], FP, kind="ExternalInput").ap()
    id_d = nc.dram_tensor("ident", [128, 128], FP, kind="ExternalInput").ap()
    y_d = nc.dram_tensor("y", [C, R], FP, kind="ExternalOutput").ap()

    with tile.TileContext(nc, trace_sim=TRACE_SIM) as tc:
        with tc.tile_pool(name="const", bufs=1) as constp, \
             tc.tile_pool(name="pers", bufs=1) as persp, \
             tc.tile_pool(name="dram", bufs=1, space="DRAM") as dramp:

            # ---------------- constants / inputs ----------------
            wqk_sb = [constp.tile([128, 2 * NH * DK], BF, name=f"wqk{cc}")
                      for cc in range(2)]
            wv_sb = [constp.tile([128, NH * DV], BF, name=f"wv{cc}")
                     for cc in range(2)]
            wp_sb = constp.tile([128, NH * C], BF)
            eb_sb = constp.tile([128, NH * 3 * N], BF)
            g1c_sb = constp.tile([128, 12], FP)
            b1c_sb = constp.tile([128, 12], FP)
            id_sb = constp.tile([128, 128], FP)
            xt_sb = [persp.tile([128, R], BF, name=f"xt{cc}") for cc in range(2)]
            ones_bf = constp.tile([128, 1], BF)
            nc.vector.memset(ones_bf[:], 1.0)
            onesrow_bf = constp.tile([1, 128], BF)
            nc.vector.memset(onesrow_bf[:], 1.0)

            # persistent
            qk_sb = [persp.tile([128, R], BF, name=f"qk{hc}") for hc in range(4)]
            alpha1 = persp.tile([128, 12], FP)
            beta1 = persp.tile([128, 12], FP)
            stats = persp.tile([128, 768], FP)
            statsg = persp.tile([128, 768], FP)
            gram_bf = persp.tile([128, 512], BF)
            meanx_bf = persp.tile([128, 2], BF)

            # W blocks for stats matmuls: (tiles, col0, width, flat-offset)
            wblocks = [(wqk_sb, 0, 512, 0),
                       (wv_sb, 0, 512, 512),
                       (wv_sb, 512, 512, 1024)]

            # ---------------- Phase A ----------------
            with tc.tile_pool(name="pA", bufs=2, space="PSUM") as pA, \
                 tc.tile_pool(name="sA", bufs=1) as sA:
                mean1s = sA.tile([128, 12], FP, tag="m1")
                vars_ = sA.tile([128, 12], FP, tag="vf")
                tmps = sA.tile([128, 12], FP, tag="tf")
                rstds = sA.tile([128, 12], FP, tag="rf")
                p_sb = [sA.tile([128, 1536], BF, tag=f"pp{c2}", name=f"p_sb{c2}")
                        for c2 in range(2)]
                with tc.tile_pool(name="xrp", bufs=1) as xrp, \
                     tc.tile_pool(name="pG", bufs=1, space="PSUM") as pG:
                    xr_sb = xrp.tile([128, 40 * C], BF)
                    for xh in range(4):
                        nc.sync.dma_start(
                            xr_sb[:, xh * 10 * C:(xh + 1) * 10 * C],
                            xr_d[:, xh * 10 * C:(xh + 1) * 10 * C])
                    nc.vector.memset(stats[:, 512:768], 0.0)
                    for cc in range(2):
                        nc.sync.dma_start(xt_sb[cc][:],
                                          xt_d[cc * 128:(cc + 1) * 128, :])
                        nc.sync.dma_start(wqk_sb[cc][:],
                                          wqk_d[cc * 128:(cc + 1) * 128, :])
                        nc.sync.dma_start(wv_sb[cc][:],
                                          wv_d[cc * 128:(cc + 1) * 128, :])
                    nc.scalar.dma_start(wp_sb[:], wp_d[:])
                    nc.scalar.dma_start(eb_sb[:], eb_d[:])
                    nc.scalar.dma_start(g1c_sb[:], g1c_d[:])
                    nc.scalar.dma_start(b1c_sb[:], b1c_d[:])
                    nc.scalar.dma_start(id_sb[:], id_d[:])
                    gps = [pG.tile([128, C], FP, tag=f"g{cc}", name=f"gps{cc}")
                           for cc in range(2)]
                    csp = pG.tile([1, C], FP, tag="cs", name="csp")
                    for rb in range(40):
                        nc.tensor.matmul(
                            gps[0][:],
                            xr_sb[:, rb * C: rb * C + 128],
                            xr_sb[:, rb * C:(rb + 1) * C],
                            start=(rb == 0), stop=(rb == 39))
                        nc.tensor.matmul(
                            gps[1][:, 128:256],
                            xr_sb[:, rb * C + 128: rb * C + 256],
                            xr_sb[:, rb * C + 128:(rb + 1) * C],
                            start=(rb == 0), stop=(rb == 39))
                        nc.tensor.matmul(
                            csp[:], ones_bf[:], xr_sb[:, rb * C:(rb + 1) * C],
                            start=(rb == 0), stop=(rb == 39))
                    nc.vector.tensor_copy(stats[:, 0:256], gps[0][:])
                    nc.vector.memset(stats[:, 256:384], 0.0)
                    nc.vector.tensor_copy(stats[:, 384:512], gps[1][:, 128:256])
                    nc.vector.tensor_copy(stats[0:1, 512:768], csp[:])
                bi = dramp.tile([128, 768], FP, tag="bi")
                bo = dramp.tile([128, 768], FP, tag="bo")
                nc.gpsimd.dma_start(bi[:], stats[:])
                nc.gpsimd.collective_compute(
                    "AllReduce", OP.add,
                    replica_groups=[list(range(NCORES))],
                    ins=[bi.opt()], outs=[bo.opt()])
                nc.gpsimd.dma_start(statsg[:], bo[:])

                # q/k matmuls (independent of stats; overlap the collective)
                for hc in range(4):
                    for rb in range(10):
                        pq = pA.tile([128, 512], FP, tag="pq")
                        for cc in range(2):
                            nc.tensor.matmul(
                                pq[:], wqk_sb[cc][:, hc * 128:(hc + 1) * 128],
                                xt_sb[cc][:, rb * 512:(rb + 1) * 512],
                                start=(cc == 0), stop=(cc == 1))
                        nc.scalar.copy(qk_sb[hc][:, rb * 512:(rb + 1) * 512],
                                       pq[:])

                # ---- BN1 stats math (gated on allreduce) ----
                with tc.tile_pool(name="pS1", bufs=1, space="PSUM") as pS1:
                    nc.gpsimd.tensor_copy(gram_bf[:, 0:128], statsg[:, 0:128])
                    nc.gpsimd.tensor_scalar(gram_bf[:, 128:256],
                                            statsg[:, 128:256], 2.0, None, OP.mult)
                    nc.gpsimd.tensor_copy(gram_bf[:, 384:512], statsg[:, 384:512])
                    for cc in range(2):
                        tpm = pS1.tile([128, 2], FP, tag="tp", bufs=1)
                        nc.tensor.transpose(
                            tpm[:, 0:1],
                            statsg[0:1, 512 + cc * 128:512 + (cc + 1) * 128],
                            id_sb[0:1, 0:1])
                        nc.vector.tensor_scalar(meanx_bf[:, cc:cc + 1], tpm[:, 0:1],
                                                1.0 / NT, None, OP.mult)
                    # mean1 [128,12] direct: out[:, hc] = sum_c W[c, hc-dims] mean_x[c]
                    m1p = pS1.tile([128, 12], FP, tag="m1p")
                    wcols = [(wqk_sb, 0), (wqk_sb, 128), (wqk_sb, 256), (wqk_sb, 384),
                             (wv_sb, 0), (wv_sb, 128), (wv_sb, 256), (wv_sb, 384),
                             (wv_sb, 512), (wv_sb, 640), (wv_sb, 768), (wv_sb, 896)]
                    for hc, (wt, c0) in enumerate(wcols):
                        for cc in range(2):
                            nc.tensor.matmul(m1p[:, hc:hc + 1],
                                             wt[cc][:, c0:c0 + 128],
                                             meanx_bf[:, cc:cc + 1],
                                             start=(cc == 0), stop=(cc == 1))
                    nc.vector.tensor_copy(mean1s[:], m1p[:])
                    # E2 decomposition using symmetric Gram:
                    # P0 = W0 .* (G00 W0); P1 = W1 .* (G11 W1 + 2 G01^T W0)
                    # where G01^T W0 is computed as M01[c2-chunk1, h] via
                    # stationary G01 [c1-part, c2-cols].
                    for (wt, c0, w, f0) in wblocks:
                        mm = pS1.tile([128, 512], FP, tag="mm", bufs=3)
                        nc.tensor.matmul(mm[:], gram_bf[:, 0:128],
                                         wt[0][:, c0:c0 + w])
                        nc.vector.tensor_tensor(p_sb[0][:, f0:f0 + w], mm[:],
                                                wt[0][:, c0:c0 + w], OP.mult)
                        mm2 = pS1.tile([128, 512], FP, tag="mm", bufs=3)
                        nc.tensor.matmul(mm2[:], gram_bf[:, 384:512],
                                         wt[1][:, c0:c0 + w], start=True, stop=False)
                        nc.tensor.matmul(mm2[:], gram_bf[:, 128:256],
                                         wt[0][:, c0:c0 + w], start=False, stop=True)
                        nc.vector.tensor_tensor(p_sb[1][:, f0:f0 + w], mm2[:],
                                                wt[1][:, c0:c0 + w], OP.mult)
                    # E2 [128,12] direct: out[:, hc] = sum_c2 P[c2, hc-dims]
                    e2p = pS1.tile([128, 12], FP, tag="e2p")
                    for hc in range(12):
                        for c2 in range(2):
                            nc.tensor.matmul(e2p[:, hc:hc + 1],
                                             p_sb[c2][:, hc * 128:(hc + 1) * 128],
                                             ones_bf[:],
                                             start=(c2 == 0), stop=(c2 == 1))
                    # var = E2/NT - mean1^2 + eps ; rstd = exp(-0.5 ln(var))
                    nc.vector.tensor_tensor(tmps[:], mean1s[:], mean1s[:], OP.mult)
                    nc.vector.tensor_scalar(tmps[:], tmps[:], -1.0, EPS, OP.mult,
                                            OP.add)
                    nc.vector.scalar_tensor_tensor(vars_[:], e2p[:], 1.0 / NT,
                                                   tmps[:], OP.mult, OP.add)
                    nc.scalar.activation(tmps[:], vars_[:], AF.Ln)
                    nc.scalar.activation(rstds[:], tmps[:], AF.Exp, scale=-0.5)
                    nc.vector.tensor_tensor(alpha1[:], g1c_sb[:], rstds[:], OP.mult)
                    nc.vector.tensor_tensor(tmps[:], mean1s[:], alpha1[:], OP.mult)
                    nc.vector.tensor_tensor(beta1[:], b1c_sb[:], tmps[:], OP.subtract)

            if DBG_STOP == "A":
                for hc in range(4):
                    nc.vector.tensor_scalar(
                        qk_sb[hc][:], qk_sb[hc][:],
                        alpha1[:, hc:hc + 1], beta1[:, hc:hc + 1], OP.mult, OP.add)
                with tc.tile_pool(name="dbg", bufs=2) as dbgp:
                    for i, hc in enumerate((0, 2)):
                        for rb in range(10):
                            dq = dbgp.tile([128, 512], FP, tag="dq", name="dq")
                            nc.vector.tensor_copy(
                                dq[:], qk_sb[hc][:, rb * 512:(rb + 1) * 512])
                            nc.sync.dma_start(
                                y_d[i * 128:(i + 1) * 128, rb * 512:(rb + 1) * 512],
                                dq[:])
            else:
                # ---------- attention + output, pair-streamed ----------
                with tc.tile_pool(name="pSm", bufs=S_BUFS, space="PSUM") as pSm, \
                     tc.tile_pool(name="pAv", bufs=AV_BUFS, space="PSUM") as pAv, \
                     tc.tile_pool(name="pVm", bufs=VMM_BUFS, space="PSUM") as pVm, \
                     tc.tile_pool(name="pPj", bufs=PROJ_BUFS, space="PSUM") as pPj, \
                     tc.tile_pool(name="etp", bufs=ET_BUFS) as etp, \
                     tc.tile_pool(name="vtp", bufs=3) as vtp, \
                     tc.tile_pool(name="gpp", bufs=10) as gpp, \
                     tc.tile_pool(name="gpr", bufs=GP_BUFS) as gpr, \
                     tc.tile_pool(name="ysp", bufs=2) as ysp, \
                     tc.tile_pool(name="rrp", bufs=6) as rrp:
                    # v m-chunk windows in a pair: (name, row0-in-pair, width)
                    VCH = [("c0", 0, 128), ("c1", 128, 128),
                           ("c2", N, 128), ("c3", N + 128, 128)]
                    g_pairs = {}
                    for pair in range(NPAIR):
                        r0 = pair * PW
                        for hc in range(4):
                            nc.gpsimd.tensor_scalar(
                                qk_sb[hc][:, r0:r0 + PW], qk_sb[hc][:, r0:r0 + PW],
                                alpha1[:, hc:hc + 1], beta1[:, hc:hc + 1],
                                OP.mult, OP.add)
                        # v matmuls -> vT chunk tiles [m, 8*(128+1)] with alpha fold
                        vts = {}
                        for (cn, rr, wdt) in VCH:
                            vt = vtp.tile([128, NH * (DV + 1)], BF, tag=f"vt{cn}",
                                          name=f"vt{cn}")
                            vts[cn] = vt
                            vt3 = vt.rearrange("p (h x) -> p h x", x=DV + 1)
                            for dh in range(2):
                                vp = pVm.tile([128, 512], FP, tag="vp")
                                for cc in range(2):
                                    nc.tensor.matmul(
                                        vp[:], xt_sb[cc][:, r0 + rr:r0 + rr + wdt],
                                        wv_sb[cc][:, dh * 512:(dh + 1) * 512],
                                        start=(cc == 0), stop=(cc == 1))
                                nc.vector.tensor_copy(
                                    vt3[:, dh * 4:dh * 4 + 4, 0:128], vp[:])
                            nc.gpsimd.memset(vt[:, 128::129], 1.0)
                        vt = vtp.tile([128, NH * (DV + 1)], BF, tag="vt4", name="vt4")
                        vts["c4"] = vt
                        vt3 = vt.rearrange("p (h x) -> p h x", x=DV + 1)
                        for dh in range(2):
                            vp = pVm.tile([128, 512], FP, tag="vp")
                            for (bb, orow) in ((0, 0), (1, 64)):
                                for cc in range(2):
                                    nc.tensor.matmul(
                                        vp[orow:orow + 64, :],
                                        xt_sb[cc][:, r0 + bb * N + 256:
                                                 r0 + bb * N + 320],
                                        wv_sb[cc][:, dh * 512:(dh + 1) * 512],
                                        start=(cc == 0), stop=(cc == 1))
                            nc.vector.tensor_copy(
                                vt3[:, dh * 4:dh * 4 + 4, 0:128], vp[:])
                        nc.gpsimd.memset(vt[:, 128::129], 1.0)

                        g_pair = gpr.tile([128, NH * PW], BF, tag="gp", name="g_pair")
                        g_pairs[pair] = g_pair
                        for h in range(NH):
                            qc, qr = h // 4, 32 * (h % 4)
                            q0 = qk_sb[qc][qr:qr + 32, r0:r0 + N]
                            q1 = qk_sb[qc][qr:qr + 32, r0 + N:r0 + PW]
                            k0 = qk_sb[2 + qc][qr:qr + 32, r0:r0 + N]
                            k1 = qk_sb[2 + qc][qr:qr + 32, r0 + N:r0 + PW]
                            sdefs = [("s0", k0[:, 0:128], q0),
                                     ("s1", k0[:, 128:256], q0),
                                     ("s2", k1[:, 0:128], q1),
                                     ("s3", k1[:, 128:256], q1)]
                            ets = []
                            for (tg, kap, qap) in sdefs:
                                sp = pSm.tile([128, N], FP, tag="s", name="sp")
                                nc.tensor.matmul(sp[:], kap, qap,
                                                 tile_position=(qr, 0))
                                et = etp.tile([128, N], BF, tag=tg, name="et")
                                nc.scalar.activation(et[:], sp[:], AF.Exp, scale=SCALE)
                                ets.append(et)
                            sp = pSm.tile([128, N], FP, tag="s", name="sp4")
                            nc.tensor.matmul(sp[0:64, :], k0[:, 256:320], q0,
                                             tile_position=(qr, 0))
                            nc.tensor.matmul(sp[64:128, :], k1[:, 256:320], q1,
                                             tile_position=(qr, 64))
                            et4 = etp.tile([128, N], BF, tag="s4", name="et4")
                            nc.scalar.activation(et4[:], sp[:], AF.Exp, scale=SCALE)
                            ets.append(et4)
                            ebmap = [0, 1, 0, 1, 2]
                            for i, et in enumerate(ets):
                                ebs = eb_sb[:, (h * 3 + ebmap[i]) * N:
                                            (h * 3 + ebmap[i] + 1) * N]
                                if i < EBM_POOL:
                                    nc.gpsimd.tensor_tensor(et[:], et[:], ebs, OP.mult)
                                else:
                                    nc.vector.tensor_tensor(et[:], et[:], ebs, OP.mult)
                            avA = pAv.tile([128, 3 * 129], FP, tag="av", name="avA")
                            avB = pAv.tile([128, 3 * 129], FP, tag="av", name="avB")
                            regions = [(avA, 0, 0, 0), (avA, 129, 0, 1),
                                       (avB, 0, 1, 0), (avB, 129, 1, 1)]
                            vmap = [["c0", "c1", "c4"], ["c2", "c3", "c4"]]
                            emap = [[0, 1, 4], [2, 3, 4]]
                            for (dst, c0_, bb, ncx) in regions:
                                for mc in range(3):
                                    et = ets[emap[bb][mc]]
                                    vtt = vts[vmap[bb][mc]]
                                    if mc == 2:
                                        sr = bb * 64
                                        stp = et[sr:sr + 64, ncx * 128:ncx * 128 + 128]
                                        mvp = vtt[sr:sr + 64, h * 129:(h + 1) * 129]
                                    else:
                                        stp = et[:, ncx * 128:ncx * 128 + 128]
                                        mvp = vtt[:, h * 129:(h + 1) * 129]
                                    nc.tensor.matmul(dst[:, c0_:c0_ + 129], stp, mvp,
                                                     start=(mc == 0), stop=(mc == 2))
                            for bb in range(2):
                                orow = bb * 64
                                for mc in range(3):
                                    et = ets[emap[bb][mc]]
                                    vtt = vts[vmap[bb][mc]]
                                    if mc == 2:
                                        sr = bb * 64
                                        stp = et[sr:sr + 64, 256:320]
                                        mvp = vtt[sr:sr + 64, h * 129:(h + 1) * 129]
                                    else:
                                        stp = et[:, 256:320]
                                        mvp = vtt[:, h * 129:(h + 1) * 129]
                                    nc.tensor.matmul(
                                        avA[orow:orow + 64, 258:387], stp, mvp,
                                        start=(mc == 0), stop=(mc == 2))
                            rA = rrp.tile([128, 3], FP, tag="rA", name="rA")
                            rB = rrp.tile([128, 2], FP, tag="rB", name="rB")
                            nc.vector.reciprocal_approx_fast(rA[:], avA[:, 128::129])
                            nc.vector.reciprocal_approx_fast(rB[:],
                                                             avB[:, 128:300:129])
                            ddefs = [(avA, 0, rA, 0, 0), (avA, 129, rA, 1, 128),
                                     (avB, 0, rB, 0, N), (avB, 129, rB, 1, N + 128)]
                            for (src, c0_, rr_, ri, gc) in ddefs:
                                gp = gpp.tile([128, 128], BF, tag="gpre", name="gp")
                                nc.vector.tensor_scalar(
                                    gp[:], src[:, c0_:c0_ + 128], rr_[:, ri:ri + 1],
                                    None, OP.mult)
                                nc.sync.dma_start_transpose(
                                    g_pair[:, h * PW + gc:h * PW + gc + 128], gp[:])
                            gp = gpp.tile([128, 128], BF, tag="gpre", name="gp4")
                            nc.vector.tensor_scalar(
                                gp[:], avA[:, 258:386], rA[:, 2:3],
                                None, OP.mult)
                            nc.sync.dma_start_transpose(
                                g_pair[:, h * PW + 256:h * PW + 320], gp[0:64, :])
                            nc.sync.dma_start_transpose(
                                g_pair[:, h * PW + N + 256:h * PW + N + 320],
                                gp[64:128, :])
                        for h in range(NH):
                            nc.gpsimd.tensor_scalar(
                                g_pair[:, h * PW:(h + 1) * PW],
                                g_pair[:, h * PW:(h + 1) * PW],
                                alpha1[:, 4 + h:5 + h], beta1[:, 4 + h:5 + h],
                                OP.mult, OP.add)
                        flush = ((pair % 2 == 1 and pair < 6) or pair >= 6)
                        if flush:
                            plist = [pair] if pair >= 6 else [pair - 1, pair]
                            for p2 in plist:
                                nc.scalar.activation(g_pairs[p2][:], g_pairs[p2][:],
                                                     AF.Gelu)
                            for p2 in plist:
                                gp2 = g_pairs.pop(p2)
                                rr0 = p2 * PW
                                for cc in range(2):
                                    yst = ysp.tile([128, PW], FP, tag=f"y{cc}",
                                                   name="yst")
                                    for half in range(2):
                                        if p2 == NPAIR - 1 and (cc + half) % 2 == 1:
                                            pp = pAv.tile([128, 3 * 129], FP,
                                                          tag="av", name="ppav")
                                            pp = pp[:, 0:N]
                                        else:
                                            pp = pPj.tile([128, N], FP, tag="pp")
                                        for h in range(NH):
                                            nc.tensor.matmul(
                                                pp[:],
                                                wp_sb[:, h * C + cc * 128:
                                                      h * C + cc * 128 + 128],
                                                gp2[:, h * PW + half * N:
                                                    h * PW + (half + 1) * N],
                                                start=(h == 0), stop=(h == NH - 1))
                                        nc.vector.tensor_copy(
                                            yst[:, half * N:(half + 1) * N], pp[:])
                                    nc.sync.dma_start(
                                        y_d[cc * 128:(cc + 1) * 128, rr0:rr0 + PW],
                                        yst[:])

    nc.compile()
    return nc


_PROG = None


def _get_prog():
    global _PROG
    if _PROG is None:
        _PROG = build_program()
    return _PROG


def _host_prep(x, Wqkv, g1, b1, ab, Wproj, g2, b2, idxs):
    x = np.asarray(x, dtype=np.float32)
    Wqkv = np.asarray(Wqkv, dtype=np.float32)
    Wproj = np.asarray(Wproj, dtype=np.float32)
    g1 = np.asarray(g1, np.float32)
    b1 = np.asarray(b1, np.float32)
    idxs = np.asarray(idxs)
    qrows = np.concatenate([np.arange(h * 192, h * 192 + 32) for h in range(NH)])
    krows = np.concatenate([np.arange(h * 192 + 32, h * 192 + 64) for h in range(NH)])
    vrows = np.concatenate([np.arange(h * 192 + 64, h * 192 + 192) for h in range(NH)])
    wqk = np.ascontiguousarray(Wqkv[np.concatenate([qrows, krows]), :].T).astype(
        ml_dtypes.bfloat16)                                     # (256, 512)
    wv = np.ascontiguousarray(Wqkv[vrows, :].T).astype(ml_dtypes.bfloat16)
    wp = np.ascontiguousarray(
        Wproj.reshape(C, NH, DV).transpose(2, 1, 0).reshape(DV, NH * C)).astype(
        ml_dtypes.bfloat16)                                     # (128, 2048)
    perm = np.concatenate([qrows, krows, vrows])
    g1c = np.ascontiguousarray(g1[perm].reshape(12, 128).T)
    b1c = np.ascontiguousarray(b1[perm].reshape(12, 128).T)
    ebm = np.exp(np.asarray(ab, np.float32))[:, idxs]           # (8, 320, 320)
    ebp = np.zeros((128, NH * 3 * N), np.float32)
    for h in range(NH):
        ebp[:, (h * 3) * N:(h * 3 + 1) * N] = ebm[h, 0:128, :]
        ebp[:, (h * 3 + 1) * N:(h * 3 + 2) * N] = ebm[h, 128:256, :]
        ebp[0:64, (h * 3 + 2) * N:(h * 3 + 3) * N] = ebm[h, 256:320, :]
        ebp[64:128, (h * 3 + 2) * N:(h * 3 + 3) * N] = ebm[h, 256:320, :]
    common = {
        "wqk": wqk, "wv": wv, "wp": wp,
        "eb": ebp.astype(ml_dtypes.bfloat16),
        "g1c": g1c, "b1c": b1c,
        "ident": np.eye(128, dtype=np.float32),
    }
    in_maps = []
    for c in range(NCORES):
        xc = np.ascontiguousarray(x[c * BL:(c + 1) * BL].reshape(R, C))
        m = dict(common)
        m["xr"] = np.ascontiguousarray(
            xc.reshape(40, 128, C).transpose(1, 0, 2).reshape(128, 40 * C)).astype(
            ml_dtypes.bfloat16)
        m["xt"] = np.ascontiguousarray(xc.T).astype(ml_dtypes.bfloat16)
        in_maps.append(m)
    return in_maps


def _postprocess(res, g2, b2):
    ys = [np.asarray(res.results[c]["y"], np.float32) for c in range(NCORES)]
    z = np.concatenate(
        [y.T.reshape(BL, N, C) for y in ys], axis=0)            # (128, 320, 256)
    zf = z.reshape(-1, C).astype(np.float64)
    m = zf.mean(axis=0)
    v = zf.var(axis=0)
    out = (z - m.astype(np.float32)) * (1.0 / np.sqrt(v + EPS)).astype(np.float32) \
        * np.asarray(g2, np.float32) + np.asarray(b2, np.float32)
    return out.astype(np.float32)


def _run(in_maps, g2, b2, trace=False):
    nc = _get_prog()
    res = run_bass_kernel_spmd(nc, in_maps, core_ids=list(range(NCORES)),
                               trace=trace)
    return _postprocess(res, g2, b2), res


def kernel(**inputs):
    out, _ = _run(_host_prep(**inputs), inputs["g2"], inputs["b2"])
    return out


def run_raw(**inputs):
    """Return raw per-core y tensors (for debugging)."""
    nc = _get_prog()
    res = run_bass_kernel_spmd(nc, _host_prep(**inputs),
                               core_ids=list(range(NCORES)))
    return [np.asarray(res.results[c]["y"], np.float32) for c in range(NCORES)]


# revision 4
# speedup vs baseline: 1.0170x; 1.0005x over previous
"""Trainium2 Bass kernel v2 for nn_Attention (qkv+BN1 -> biased softmax attn -> gelu -> proj; BN2 on host).

Data-parallel over batch B=128 across 8 cores (16 batches = 8 "pairs" of 640 rows each).

Key structure (per core):
  - Host sends xT [256,5120] bf16 (moving operand for q/k/v matmuls) and
    x_rows packed [128, 40*256] bf16 (stationary for the x Gram matrix).
  - BN1 stats via Gram: var(qkv_h) = W_h (Sigma_x) W_h^T - mean1^2, with
    Sigma_x = sum_r x x^T allreduced EARLY (overlaps q/k/v matmuls).
  - Attention S[m-part, n-free] per (head, pair); the two 64-row mc2 chunks of
    a batch-pair share one PSUM tile (5 exps per (h,pair) instead of 6).
  - et = exp(scale*S) (Act) then *= exp(bias) gathered on host (DVE/Pool).
  - AV flipped: out[n, d] = sum_m et[m,n] v[m,d] with v stored [m, 8*(128+1)]
    (interleaved ones columns) so out col 128 = softmax denominator Z[n].
  - divide+BN1-v-beta: g_pre = (av * (1/Z)[n-part]) + beta_bc (Pool stt);
    alpha_v folded into the v PSUM->SBUF copy; then DMA-transpose to g_pair
    [d-part, row-free]; gelu per pair; proj -> y [c-part, row] fp32.
    BN2 (training-mode batchnorm, final linear op) done on HOST.
"""
import os
import numpy as np
import ml_dtypes

import concourse.bass as bass
import concourse.tile as tile
from concourse import bacc, mybir
from concourse.bass_utils import run_bass_kernel_spmd

NCORES = 8
TRACE_SIM = os.environ.get("KERN_TRACE_SIM", "") == "1"
DBG_STOP = os.environ.get("KERN_STOP", "")
B, N, C = 128, 320, 256
NH, DK, DV = 8, 32, 128
BL = B // NCORES             # 16 batches/core
R = BL * N                   # 5120 rows/core
NPAIR = BL // 2              # 8 pairs
PW = 2 * N                   # 640 rows/pair
NT = B * N                   # 40960 global rows
EPS = 1e-5
SCALE = DK ** -0.5
FP = mybir.dt.float32
BF = mybir.dt.bfloat16
AF = mybir.ActivationFunctionType
OP = mybir.AluOpType

S_BUFS = int(os.environ.get("K2_S_BUFS", "4"))
AV_BUFS = int(os.environ.get("K2_AV_BUFS", "2"))
VMM_BUFS = int(os.environ.get("K2_VMM_BUFS", "1"))
PROJ_BUFS = int(os.environ.get("K2_PROJ_BUFS", "1"))
ET_BUFS = int(os.environ.get("K2_ET_BUFS", "8"))
GP_BUFS = int(os.environ.get("K2_GP_BUFS", "3"))
EBM_POOL = int(os.environ.get("K2_EBM_POOL", "5"))  # of 5 eb-mults on Pool


def build_program():
    nc = bacc.Bacc("TRN2", target_bir_lowering=False, debug=False,
                   enable_asserts=False, num_devices=NCORES)
    xr_d = nc.dram_tensor("xr", [128, 40 * C], BF, kind="ExternalInput").ap()
    xt_d = nc.dram_tensor("xt", [C, R], BF, kind="ExternalInput").ap()
    wqk_d = nc.dram_tensor("wqk", [C, 2 * NH * DK], BF, kind="ExternalInput").ap()
    wv_d = nc.dram_tensor("wv", [C, NH * DV], BF, kind="ExternalInput").ap()
    wp_d = nc.dram_tensor("wp", [DV, NH * C], BF, kind="ExternalInput").ap()
    eb_d = nc.dram_tensor("eb", [128, NH * 3 * N], BF, kind="ExternalInput").ap()
    g1c_d = nc.dram_tensor("g1c", [128, 12], FP, kind="ExternalInput").ap()
    b1c_d = nc.dram_tensor("b1c", [128, 12], FP, kind="ExternalInput").ap()
    id_d = nc.dram_tensor("ident", [128, 128], FP, kind="ExternalInput").ap()
    y_d = nc.dram_tensor("y", [C, R], FP, kind="ExternalOutput").ap()

    with tile.TileContext(nc, trace_sim=TRACE_SIM) as tc:
        with tc.tile_pool(name="const", bufs=1) as constp, \
             tc.tile_pool(name="pers", bufs=1) as persp, \
             tc.tile_pool(name="dram", bufs=1, space="DRAM") as dramp:

            # ---------------- constants / inputs ----------------
            wqk_sb = [constp.tile([128, 2 * NH * DK], BF, name=f"wqk{cc}")
                      for cc in range(2)]
            wv_sb = [constp.tile([128, NH * DV], BF, name=f"wv{cc}")
                     for cc in range(2)]
            wp_sb = constp.tile([128, NH * C], BF)
            eb_sb = constp.tile([128, NH * 3 * N], BF)
            g1c_sb = constp.tile([128, 12], FP)
            b1c_sb = constp.tile([128, 12], FP)
            id_sb = constp.tile([128, 128], FP)
            xt_sb = [persp.tile([128, R], BF, name=f"xt{cc}") for cc in range(2)]
            ones_bf = constp.tile([128, 1], BF)
            nc.vector.memset(ones_bf[:], 1.0)
            eps_c = constp.tile([128, 1], FP)
            nc.vector.memset(eps_c[:], EPS)
            onesrow_bf = constp.tile([1, 128], BF)
            nc.vector.memset(onesrow_bf[:], 1.0)

            # persistent
            qk_sb = [persp.tile([128, R], BF, name=f"qk{hc}") for hc in range(4)]
            alpha1 = persp.tile([128, 12], FP)
            beta1 = persp.tile([128, 12], FP)
            stats = persp.tile([128, 768], FP)
            statsg = persp.tile([128, 768], FP)
            gram_bf = persp.tile([128, 512], BF)
            meanx_bf = persp.tile([128, 2], BF)

            # W blocks for stats matmuls: (tiles, col0, width, flat-offset)
            wblocks = [(wqk_sb, 0, 512, 0),
                       (wv_sb, 0, 512, 512),
                       (wv_sb, 512, 512, 1024)]

            # ---------------- Phase A ----------------
            with tc.tile_pool(name="pA", bufs=2, space="PSUM") as pA, \
                 tc.tile_pool(name="sA", bufs=1) as sA:
                mean1s = sA.tile([128, 12], FP, tag="m1")
                vars_ = sA.tile([128, 12], FP, tag="vf")
                tmps = sA.tile([128, 12], FP, tag="tf")
                rstds = sA.tile([128, 12], FP, tag="rf")
                p_sb = [sA.tile([128, 1536], BF, tag=f"pp{c2}", name=f"p_sb{c2}")
                        for c2 in range(2)]
                with tc.tile_pool(name="xrp", bufs=1) as xrp, \
                     tc.tile_pool(name="pG", bufs=1, space="PSUM") as pG:
                    xr_sb = xrp.tile([128, 40 * C], BF)
                    for xh in (0, 1):
                        nc.sync.dma_start(
                            xr_sb[:, xh * 10 * C:(xh + 1) * 10 * C],
                            xr_d[:, xh * 10 * C:(xh + 1) * 10 * C])
                    for xh in (2, 3):
                        nc.scalar.dma_start(
                            xr_sb[:, xh * 10 * C:(xh + 1) * 10 * C],
                            xr_d[:, xh * 10 * C:(xh + 1) * 10 * C])
                    nc.vector.memset(stats[:, 512:768], 0.0)
                    for cc in range(2):
                        nc.sync.dma_start(xt_sb[cc][:],
                                          xt_d[cc * 128:(cc + 1) * 128, :])
                        nc.sync.dma_start(wqk_sb[cc][:],
                                          wqk_d[cc * 128:(cc + 1) * 128, :])
                        nc.sync.dma_start(wv_sb[cc][:],
                                          wv_d[cc * 128:(cc + 1) * 128, :])
                    nc.scalar.dma_start(wp_sb[:], wp_d[:])
                    nc.scalar.dma_start(eb_sb[:], eb_d[:])
                    nc.scalar.dma_start(g1c_sb[:], g1c_d[:])
                    nc.scalar.dma_start(b1c_sb[:], b1c_d[:])
                    nc.scalar.dma_start(id_sb[:], id_d[:])
                    gps = [pG.tile([128, C], FP, tag=f"g{cc}", name=f"gps{cc}")
                           for cc in range(2)]
                    csp = pG.tile([1, C], FP, tag="cs", name="csp")
                    rbord = ([rb for x in (0, 2) for rb in range(x * 10, x * 10 + 10)]
                             + [rb for x in (1, 3) for rb in range(x * 10, x * 10 + 10)])
                    for i, rb in enumerate(rbord):
                        fl = (i == 0)
                        ll = (i == 39)
                        nc.tensor.matmul(
                            gps[0][:],
                            xr_sb[:, rb * C: rb * C + 128],
                            xr_sb[:, rb * C:(rb + 1) * C],
                            start=fl, stop=ll)
                        nc.tensor.matmul(
                            gps[1][:, 128:256],
                            xr_sb[:, rb * C + 128: rb * C + 256],
                            xr_sb[:, rb * C + 128:(rb + 1) * C],
                            start=fl, stop=ll)
                        nc.tensor.matmul(
                            csp[:], ones_bf[:], xr_sb[:, rb * C:(rb + 1) * C],
                            start=fl, stop=ll)
                    nc.vector.tensor_copy(stats[:, 0:256], gps[0][:])
                    nc.vector.memset(stats[:, 256:384], 0.0)
                    nc.vector.tensor_copy(stats[:, 384:512], gps[1][:, 128:256])
                    nc.vector.tensor_copy(stats[0:1, 512:768], csp[:])
                bi = dramp.tile([128, 768], FP, tag="bi")
                bo = dramp.tile([128, 768], FP, tag="bo")
                nc.gpsimd.dma_start(bi[:], stats[:])
                nc.gpsimd.collective_compute(
                    "AllReduce", OP.add,
                    replica_groups=[list(range(NCORES))],
                    ins=[bi.opt()], outs=[bo.opt()])
                nc.gpsimd.dma_start(statsg[:], bo[:])

                # q/k matmuls (independent of stats; overlap the collective)
                for hc in range(4):
                    for rb in range(10):
                        pq = pA.tile([128, 512], FP, tag="pq")
                        for cc in range(2):
                            nc.tensor.matmul(
                                pq[:], wqk_sb[cc][:, hc * 128:(hc + 1) * 128],
                                xt_sb[cc][:, rb * 512:(rb + 1) * 512],
                                start=(cc == 0), stop=(cc == 1))
                        nc.scalar.copy(qk_sb[hc][:, rb * 512:(rb + 1) * 512],
                                       pq[:])

                # ---- BN1 stats math (gated on allreduce) ----
                with tc.tile_pool(name="pS1", bufs=1, space="PSUM") as pS1:
                    nc.gpsimd.tensor_copy(gram_bf[:, 0:128], statsg[:, 0:128])
                    nc.gpsimd.tensor_scalar(gram_bf[:, 128:256],
                                            statsg[:, 128:256], 2.0, None, OP.mult)
                    nc.gpsimd.tensor_copy(gram_bf[:, 384:512], statsg[:, 384:512])
                    for cc in range(2):
                        tpm = pS1.tile([128, 2], FP, tag="tp", bufs=1)
                        nc.tensor.transpose(
                            tpm[:, 0:1],
                            statsg[0:1, 512 + cc * 128:512 + (cc + 1) * 128],
                            id_sb[0:1, 0:1])
                        nc.vector.tensor_scalar(meanx_bf[:, cc:cc + 1], tpm[:, 0:1],
                                                1.0 / NT, None, OP.mult)
                    # mean1 [128,12] direct: out[:, hc] = sum_c W[c, hc-dims] mean_x[c]
                    m1p = pS1.tile([128, 12], FP, tag="m1p")
                    wcols = [(wqk_sb, 0), (wqk_sb, 128), (wqk_sb, 256), (wqk_sb, 384),
                             (wv_sb, 0), (wv_sb, 128), (wv_sb, 256), (wv_sb, 384),
                             (wv_sb, 512), (wv_sb, 640), (wv_sb, 768), (wv_sb, 896)]
                    for hc, (wt, c0) in enumerate(wcols):
                        for cc in range(2):
                            nc.tensor.matmul(m1p[:, hc:hc + 1],
                                             wt[cc][:, c0:c0 + 128],
                                             meanx_bf[:, cc:cc + 1],
                                             start=(cc == 0), stop=(cc == 1))
                    nc.vector.tensor_copy(mean1s[:], m1p[:])
                    # E2 decomposition using symmetric Gram:
                    # P0 = W0 .* (G00 W0); P1 = W1 .* (G11 W1 + 2 G01^T W0)
                    # where G01^T W0 is computed as M01[c2-chunk1, h] via
                    # stationary G01 [c1-part, c2-cols].
                    for (wt, c0, w, f0) in wblocks:
                        mm = pS1.tile([128, 512], FP, tag="mm", bufs=3)
                        nc.tensor.matmul(mm[:], gram_bf[:, 0:128],
                                         wt[0][:, c0:c0 + w])
                        nc.vector.tensor_tensor(p_sb[0][:, f0:f0 + w], mm[:],
                                                wt[0][:, c0:c0 + w], OP.mult)
                        mm2 = pS1.tile([128, 512], FP, tag="mm", bufs=3)
                        nc.tensor.matmul(mm2[:], gram_bf[:, 384:512],
                                         wt[1][:, c0:c0 + w], start=True, stop=False)
                        nc.tensor.matmul(mm2[:], gram_bf[:, 128:256],
                                         wt[0][:, c0:c0 + w], start=False, stop=True)
                        nc.vector.tensor_tensor(p_sb[1][:, f0:f0 + w], mm2[:],
                                                wt[1][:, c0:c0 + w], OP.mult)
                    # E2 [128,12] direct: out[:, hc] = sum_c2 P[c2, hc-dims]
                    e2p = pS1.tile([128, 12], FP, tag="e2p")
                    for hc in range(12):
                        for c2 in range(2):
                            nc.tensor.matmul(e2p[:, hc:hc + 1],
                                             p_sb[c2][:, hc * 128:(hc + 1) * 128],
                                             ones_bf[:],
                                             start=(c2 == 0), stop=(c2 == 1))
                    # var = E2/NT - mean1^2 + eps ; rstd = exp(-0.5 ln(var))
                    nc.vector.scalar_tensor_tensor(tmps[:], mean1s[:], -1.0,
                                                   mean1s[:], OP.mult, OP.mult)
                    nc.vector.scalar_tensor_tensor(vars_[:], e2p[:], 1.0 / NT,
                                                   tmps[:], OP.mult, OP.add)
                    nc.scalar.activation(tmps[:], vars_[:], AF.Ln, bias=eps_c[:])
                    nc.scalar.activation(rstds[:], tmps[:], AF.Exp, scale=-0.5)
                    nc.vector.tensor_tensor(alpha1[:], g1c_sb[:], rstds[:], OP.mult)
                    nc.vector.tensor_tensor(tmps[:], mean1s[:], alpha1[:], OP.mult)
                    nc.vector.tensor_tensor(beta1[:], b1c_sb[:], tmps[:], OP.subtract)

            if DBG_STOP == "A":
                for hc in range(4):
                    nc.vector.tensor_scalar(
                        qk_sb[hc][:], qk_sb[hc][:],
                        alpha1[:, hc:hc + 1], beta1[:, hc:hc + 1], OP.mult, OP.add)
                with tc.tile_pool(name="dbg", bufs=2) as dbgp:
                    for i, hc in enumerate((0, 2)):
                        for rb in range(10):
                            dq = dbgp.tile([128, 512], FP, tag="dq", name="dq")
                            nc.vector.tensor_copy(
                                dq[:], qk_sb[hc][:, rb * 512:(rb + 1) * 512])
                            nc.sync.dma_start(
                                y_d[i * 128:(i + 1) * 128, rb * 512:(rb + 1) * 512],
                                dq[:])
            else:
                # ---------- attention + output, pair-streamed ----------
                with tc.tile_pool(name="pSm", bufs=S_BUFS, space="PSUM") as pSm, \
                     tc.tile_pool(name="pAv", bufs=AV_BUFS, space="PSUM") as pAv, \
                     tc.tile_pool(name="pVm", bufs=VMM_BUFS, space="PSUM") as pVm, \
                     tc.tile_pool(name="pPj", bufs=PROJ_BUFS, space="PSUM") as pPj, \
                     tc.tile_pool(name="etp", bufs=ET_BUFS) as etp, \
                     tc.tile_pool(name="vtp", bufs=3) as vtp, \
                     tc.tile_pool(name="gpp", bufs=10) as gpp, \
                     tc.tile_pool(name="gpr", bufs=GP_BUFS) as gpr, \
                     tc.tile_pool(name="ysp", bufs=2) as ysp, \
                     tc.tile_pool(name="rrp", bufs=6) as rrp:
                    # v m-chunk windows in a pair: (name, row0-in-pair, width)
                    VCH = [("c0", 0, 128), ("c1", 128, 128),
                           ("c2", N, 128), ("c3", N + 128, 128)]
                    g_pairs = {}
                    for pair in range(NPAIR):
                        r0 = pair * PW
                        for hc in range(4):
                            nc.gpsimd.tensor_scalar(
                                qk_sb[hc][:, r0:r0 + PW], qk_sb[hc][:, r0:r0 + PW],
                                alpha1[:, hc:hc + 1], beta1[:, hc:hc + 1],
                                OP.mult, OP.add)
                        # v matmuls -> vT chunk tiles [m, 8*(128+1)] with alpha fold
                        vts = {}
                        for (cn, rr, wdt) in VCH:
                            vt = vtp.tile([128, NH * (DV + 1)], BF, tag=f"vt{cn}",
                                          name=f"vt{cn}")
                            vts[cn] = vt
                            vt3 = vt.rearrange("p (h x) -> p h x", x=DV + 1)
                            for dh in range(2):
                                vp = pVm.tile([128, 512], FP, tag="vp")
                                for cc in range(2):
                                    nc.tensor.matmul(
                                        vp[:], xt_sb[cc][:, r0 + rr:r0 + rr + wdt],
                                        wv_sb[cc][:, dh * 512:(dh + 1) * 512],
                                        start=(cc == 0), stop=(cc == 1))
                                nc.vector.tensor_copy(
                                    vt3[:, dh * 4:dh * 4 + 4, 0:128], vp[:])
                            nc.gpsimd.memset(vt[:, 128::129], 1.0)
                        vt = vtp.tile([128, NH * (DV + 1)], BF, tag="vt4", name="vt4")
                        vts["c4"] = vt
                        vt3 = vt.rearrange("p (h x) -> p h x", x=DV + 1)
                        for dh in range(2):
                            vp = pVm.tile([128, 512], FP, tag="vp")
                            for (bb, orow) in ((0, 0), (1, 64)):
                                for cc in range(2):
                                    nc.tensor.matmul(
                                        vp[orow:orow + 64, :],
                                        xt_sb[cc][:, r0 + bb * N + 256:
                                                 r0 + bb * N + 320],
                                        wv_sb[cc][:, dh * 512:(dh + 1) * 512],
                                        start=(cc == 0), stop=(cc == 1))
                            nc.vector.tensor_copy(
                                vt3[:, dh * 4:dh * 4 + 4, 0:128], vp[:])
                        nc.gpsimd.memset(vt[:, 128::129], 1.0)

                        g_pair = gpr.tile([128, NH * PW], BF, tag="gp", name="g_pair")
                        g_pairs[pair] = g_pair
                        for h in range(NH):
                            qc, qr = h // 4, 32 * (h % 4)
                            q0 = qk_sb[qc][qr:qr + 32, r0:r0 + N]
                            q1 = qk_sb[qc][qr:qr + 32, r0 + N:r0 + PW]
                            k0 = qk_sb[2 + qc][qr:qr + 32, r0:r0 + N]
                            k1 = qk_sb[2 + qc][qr:qr + 32, r0 + N:r0 + PW]
                            sdefs = [("s0", k0[:, 0:128], q0),
                                     ("s1", k0[:, 128:256], q0),
                                     ("s2", k1[:, 0:128], q1),
                                     ("s3", k1[:, 128:256], q1)]
                            ets = []
                            for (tg, kap, qap) in sdefs:
                                sp = pSm.tile([128, N], FP, tag="s", name="sp")
                                nc.tensor.matmul(sp[:], kap, qap,
                                                 tile_position=(qr, 0))
                                et = etp.tile([128, N], BF, tag=tg, name="et")
                                nc.scalar.activation(et[:], sp[:], AF.Exp, scale=SCALE)
                                ets.append(et)
                            sp = pSm.tile([128, N], FP, tag="s", name="sp4")
                            nc.tensor.matmul(sp[0:64, :], k0[:, 256:320], q0,
                                             tile_position=(qr, 0))
                            nc.tensor.matmul(sp[64:128, :], k1[:, 256:320], q1,
                                             tile_position=(qr, 64))
                            et4 = etp.tile([128, N], BF, tag="s4", name="et4")
                            nc.scalar.activation(et4[:], sp[:], AF.Exp, scale=SCALE)
                            ets.append(et4)
                            ebmap = [0, 1, 0, 1, 2]
                            for i, et in enumerate(ets):
                                ebs = eb_sb[:, (h * 3 + ebmap[i]) * N:
                                            (h * 3 + ebmap[i] + 1) * N]
                                if i < EBM_POOL:
                                    nc.gpsimd.tensor_tensor(et[:], et[:], ebs, OP.mult)
                                else:
                                    nc.vector.tensor_tensor(et[:], et[:], ebs, OP.mult)
                            avA = pAv.tile([128, 3 * 129], FP, tag="av", name="avA")
                            avB = pAv.tile([128, 3 * 129], FP, tag="av", name="avB")
                            regions = [(avA, 0, 0, 0), (avA, 129, 0, 1),
                                       (avB, 0, 1, 0), (avB, 129, 1, 1)]
                            vmap = [["c0", "c1", "c4"], ["c2", "c3", "c4"]]
                            emap = [[0, 1, 4], [2, 3, 4]]
                            for (dst, c0_, bb, ncx) in regions:
                                for mc in range(3):
                                    et = ets[emap[bb][mc]]
                                    vtt = vts[vmap[bb][mc]]
                                    if mc == 2:
                                        sr = bb * 64
                                        stp = et[sr:sr + 64, ncx * 128:ncx * 128 + 128]
                                        mvp = vtt[sr:sr + 64, h * 129:(h + 1) * 129]
                                    else:
                                        stp = et[:, ncx * 128:ncx * 128 + 128]
                                        mvp = vtt[:, h * 129:(h + 1) * 129]
                                    nc.tensor.matmul(dst[:, c0_:c0_ + 129], stp, mvp,
                                                     start=(mc == 0), stop=(mc == 2))
                            for bb in range(2):
                                orow = bb * 64
                                for mc in range(3):
                                    et = ets[emap[bb][mc]]
                                    vtt = vts[vmap[bb][mc]]
                                    if mc == 2:
                                        sr = bb * 64
                                        stp = et[sr:sr + 64, 256:320]
                                        mvp = vtt[sr:sr + 64, h * 129:(h + 1) * 129]
                                    else:
                                        stp = et[:, 256:320]
                                        mvp = vtt[:, h * 129:(h + 1) * 129]
                                    nc.tensor.matmul(
                                        avA[orow:orow + 64, 258:387], stp, mvp,
                                        start=(mc == 0), stop=(mc == 2))
                            rA = rrp.tile([128, 3], FP, tag="rA", name="rA")
                            rB = rrp.tile([128, 2], FP, tag="rB", name="rB")
                            nc.vector.reciprocal_approx_fast(rA[:], avA[:, 128::129])
                            nc.vector.reciprocal_approx_fast(rB[:],
                                                             avB[:, 128:300:129])
                            ddefs = [(avA, 0, rA, 0, 0), (avA, 129, rA, 1, 128),
                                     (avB, 0, rB, 0, N), (avB, 129, rB, 1, N + 128)]
                            for (src, c0_, rr_, ri, gc) in ddefs:
                                gp = gpp.tile([128, 128], BF, tag="gpre", name="gp")
                                nc.vector.tensor_scalar(
                                    gp[:], src[:, c0_:c0_ + 128], rr_[:, ri:ri + 1],
                                    None, OP.mult)
                                nc.sync.dma_start_transpose(
                                    g_pair[:, h * PW + gc:h * PW + gc + 128], gp[:])
                            gp = gpp.tile([128, 128], BF, tag="gpre", name="gp4")
                            nc.vector.tensor_scalar(
                                gp[:], avA[:, 258:386], rA[:, 2:3],
                                None, OP.mult)
                            nc.sync.dma_start_transpose(
                                g_pair[:, h * PW + 256:h * PW + 320], gp[0:64, :])
                            nc.sync.dma_start_transpose(
                                g_pair[:, h * PW + N + 256:h * PW + N + 320],
                                gp[64:128, :])
                        for h in range(NH):
                            nc.gpsimd.tensor_scalar(
                                g_pair[:, h * PW:(h + 1) * PW],
                                g_pair[:, h * PW:(h + 1) * PW],
                                alpha1[:, 4 + h:5 + h], beta1[:, 4 + h:5 + h],
                                OP.mult, OP.add)
                        flush = ((pair % 2 == 1 and pair < 6) or pair >= 6)
                        if flush:
                            plist = [pair] if pair >= 6 else [pair - 1, pair]
                            for p2 in plist:
                                nc.scalar.activation(g_pairs[p2][:], g_pairs[p2][:],
                                                     AF.Gelu)
                            for p2 in plist:
                                gp2 = g_pairs.pop(p2)
                                rr0 = p2 * PW
                                for cc in range(2):
                                    yst = ysp.tile([128, PW], FP, tag=f"y{cc}",
                                                   name="yst")
                                    for half in range(2):
                                        if p2 == NPAIR - 1 and (cc + half) % 2 == 1:
                                            pp = pAv.tile([128, 3 * 129], FP,
                                                          tag="av", name="ppav")
                                            pp = pp[:, 0:N]
                                        else:
                                            pp = pPj.tile([128, N], FP, tag="pp")
                                        for h in range(NH):
                                            nc.tensor.matmul(
                                                pp[:],
                                                wp_sb[:, h * C + cc * 128:
                                                      h * C + cc * 128 + 128],
                                                gp2[:, h * PW + half * N:
                                                    h * PW + (half + 1) * N],
                                                start=(h == 0), stop=(h == NH - 1))
                                        nc.vector.tensor_copy(
                                            yst[:, half * N:(half + 1) * N], pp[:])
                                    nc.sync.dma_start(
                                        y_d[cc * 128:(cc + 1) * 128, rr0:rr0 + PW],
                                        yst[:])

    nc.compile()
    return nc


_PROG = None


def _get_prog():
    global _PROG
    if _PROG is None:
        _PROG = build_program()
    return _PROG


def _host_prep(x, Wqkv, g1, b1, ab, Wproj, g2, b2, idxs):
    x = np.asarray(x, dtype=np.float32)
    Wqkv = np.asarray(Wqkv, dtype=np.float32)
    Wproj = np.asarray(Wproj, dtype=np.float32)
    g1 = np.asarray(g1, np.float32)
    b1 = np.asarray(b1, np.float32)
    idxs = np.asarray(idxs)
    qrows = np.concatenate([np.arange(h * 192, h * 192 + 32) for h in range(NH)])
    krows = np.concatenate([np.arange(h * 192 + 32, h * 192 + 64) for h in range(NH)])
    vrows = np.concatenate([np.arange(h * 192 + 64, h * 192 + 192) for h in range(NH)])
    wqk = np.ascontiguousarray(Wqkv[np.concatenate([qrows, krows]), :].T).astype(
        ml_dtypes.bfloat16)                                     # (256, 512)
    wv = np.ascontiguousarray(Wqkv[vrows, :].T).astype(ml_dtypes.bfloat16)
    wp = np.ascontiguousarray(
        Wproj.reshape(C, NH, DV).transpose(2, 1, 0).reshape(DV, NH * C)).astype(
        ml_dtypes.bfloat16)                                     # (128, 2048)
    perm = np.concatenate([qrows, krows, vrows])
    g1c = np.ascontiguousarray(g1[perm].reshape(12, 128).T)
    b1c = np.ascontiguousarray(b1[perm].reshape(12, 128).T)
    ebm = np.exp(np.asarray(ab, np.float32))[:, idxs]           # (8, 320, 320)
    ebp = np.zeros((128, NH * 3 * N), np.float32)
    for h in range(NH):
        ebp[:, (h * 3) * N:(h * 3 + 1) * N] = ebm[h, 0:128, :]
        ebp[:, (h * 3 + 1) * N:(h * 3 + 2) * N] = ebm[h, 128:256, :]
        ebp[0:64, (h * 3 + 2) * N:(h * 3 + 3) * N] = ebm[h, 256:320, :]
        ebp[64:128, (h * 3 + 2) * N:(h * 3 + 3) * N] = ebm[h, 256:320, :]
    common = {
        "wqk": wqk, "wv": wv, "wp": wp,
        "eb": ebp.astype(ml_dtypes.bfloat16),
        "g1c": g1c, "b1c": b1c,
        "ident": np.eye(128, dtype=np.float32),
    }
    in_maps = []
    for c in range(NCORES):
        xc = np.ascontiguousarray(x[c * BL:(c + 1) * BL].reshape(R, C))
        m = dict(common)
        m["xr"] = np.ascontiguousarray(
            xc.reshape(40, 128, C).transpose(1, 0, 2).reshape(128, 40 * C)).astype(
            ml_dtypes.bfloat16)
        m["xt"] = np.ascontiguousarray(xc.T).astype(ml_dtypes.bfloat16)
        in_maps.append(m)
    return in_maps


def _postprocess(res, g2, b2):
    ys = [np.asarray(res.results[c]["y"], np.float32) for c in range(NCORES)]
    z = np.concatenate(
        [y.T.reshape(BL, N, C) for y in ys], axis=0)            # (128, 320, 256)
    zf = z.reshape(-1, C).astype(np.float64)
    m = zf.mean(axis=0)
    v = zf.var(axis=0)
    out = (z - m.astype(np.float32)) * (1.0 / np.sqrt(v + EPS)).astype(np.float32) \
        * np.asarray(g2, np.float32) + np.asarray(b2, np.float32)
    return out.astype(np.float32)


def _run(in_maps, g2, b2, trace=False):
    nc = _get_prog()
    res = run_bass_kernel_spmd(nc, in_maps, core_ids=list(range(NCORES)),
                               trace=trace)
    return _postprocess(res, g2, b2), res


def kernel(**inputs):
    out, _ = _run(_host_prep(**inputs), inputs["g2"], inputs["b2"])
    return out


def run_raw(**inputs):
    """Return raw per-core y tensors (for debugging)."""
    nc = _get_prog()
    res = run_bass_kernel_spmd(nc, _host_prep(**inputs),
                               core_ids=list(range(NCORES)))
    return [np.asarray(res.results[c]["y"], np.float32) for c in range(NCORES)]


# revision 5
# speedup vs baseline: 1.0253x; 1.0082x over previous
"""Trainium2 Bass kernel v2 for nn_Attention (qkv+BN1 -> biased softmax attn -> gelu -> proj; BN2 on host).

Data-parallel over batch B=128 across 8 cores (16 batches = 8 "pairs" of 640 rows each).

Key structure (per core):
  - Host sends xT [256,5120] bf16 (moving operand for q/k/v matmuls) and
    x_rows packed [128, 40*256] bf16 (stationary for the x Gram matrix).
  - BN1 stats via Gram: var(qkv_h) = W_h (Sigma_x) W_h^T - mean1^2, with
    Sigma_x = sum_r x x^T allreduced EARLY (overlaps q/k/v matmuls).
  - Attention S[m-part, n-free] per (head, pair); the two 64-row mc2 chunks of
    a batch-pair share one PSUM tile (5 exps per (h,pair) instead of 6).
  - et = exp(scale*S) (Act) then *= exp(bias) gathered on host (DVE/Pool).
  - AV flipped: out[n, d] = sum_m et[m,n] v[m,d] with v stored [m, 8*(128+1)]
    (interleaved ones columns) so out col 128 = softmax denominator Z[n].
  - divide+BN1-v-beta: g_pre = (av * (1/Z)[n-part]) + beta_bc (Pool stt);
    alpha_v folded into the v PSUM->SBUF copy; then DMA-transpose to g_pair
    [d-part, row-free]; gelu per pair; proj -> y [c-part, row] fp32.
    BN2 (training-mode batchnorm, final linear op) done on HOST.
"""
import os
import numpy as np
import ml_dtypes

import concourse.bass as bass
import concourse.tile as tile
from concourse import bacc, mybir
from concourse.bass_utils import run_bass_kernel_spmd

NCORES = 8
TRACE_SIM = os.environ.get("KERN_TRACE_SIM", "") == "1"
DBG_STOP = os.environ.get("KERN_STOP", "")
B, N, C = 128, 320, 256
NH, DK, DV = 8, 32, 128
BL = B // NCORES             # 16 batches/core
R = BL * N                   # 5120 rows/core
NPAIR = BL // 2              # 8 pairs
PW = 2 * N                   # 640 rows/pair
NT = B * N                   # 40960 global rows
EPS = 1e-5
SCALE = DK ** -0.5
FP = mybir.dt.float32
BF = mybir.dt.bfloat16
AF = mybir.ActivationFunctionType
OP = mybir.AluOpType

S_BUFS = int(os.environ.get("K2_S_BUFS", "4"))
AV_BUFS = int(os.environ.get("K2_AV_BUFS", "2"))
VMM_BUFS = int(os.environ.get("K2_VMM_BUFS", "1"))
PROJ_BUFS = int(os.environ.get("K2_PROJ_BUFS", "1"))
ET_BUFS = int(os.environ.get("K2_ET_BUFS", "8"))
GP_BUFS = int(os.environ.get("K2_GP_BUFS", "3"))
EBM_POOL = int(os.environ.get("K2_EBM_POOL", "5"))  # of 5 eb-mults on Pool


def build_program():
    nc = bacc.Bacc("TRN2", target_bir_lowering=False, debug=False,
                   enable_asserts=False, num_devices=NCORES)
    xr_d = nc.dram_tensor("xr", [128, 40 * C], BF, kind="ExternalInput").ap()
    xt_d = nc.dram_tensor("xt", [C, R], BF, kind="ExternalInput").ap()
    wqk_d = nc.dram_tensor("wqk", [C, 2 * NH * DK], BF, kind="ExternalInput").ap()
    wv_d = nc.dram_tensor("wv", [C, NH * DV], BF, kind="ExternalInput").ap()
    wp_d = nc.dram_tensor("wp", [DV, NH * C], BF, kind="ExternalInput").ap()
    eb_d = nc.dram_tensor("eb", [128, NH * 3 * N], BF, kind="ExternalInput").ap()
    g1c_d = nc.dram_tensor("g1c", [128, 12], FP, kind="ExternalInput").ap()
    b1c_d = nc.dram_tensor("b1c", [128, 12], FP, kind="ExternalInput").ap()
    id_d = nc.dram_tensor("ident", [128, 128], FP, kind="ExternalInput").ap()
    y_d = nc.dram_tensor("y", [C, R], FP, kind="ExternalOutput").ap()

    with tile.TileContext(nc, trace_sim=TRACE_SIM) as tc:
        with tc.tile_pool(name="const", bufs=1) as constp, \
             tc.tile_pool(name="pers", bufs=1) as persp, \
             tc.tile_pool(name="dram", bufs=1, space="DRAM") as dramp:

            # ---------------- constants / inputs ----------------
            wqk_sb = [constp.tile([128, 2 * NH * DK], BF, name=f"wqk{cc}")
                      for cc in range(2)]
            wv_sb = [constp.tile([128, NH * DV], BF, name=f"wv{cc}")
                     for cc in range(2)]
            wp_sb = constp.tile([128, NH * C], BF)
            eb_sb = constp.tile([128, NH * 3 * N], BF)
            g1c_sb = constp.tile([128, 12], FP)
            b1c_sb = constp.tile([128, 12], FP)
            id_sb = constp.tile([128, 128], FP)
            xt_sb = [persp.tile([128, R], BF, name=f"xt{cc}") for cc in range(2)]
            ones_bf = constp.tile([128, 1], BF)
            nc.vector.memset(ones_bf[:], 1.0)
            eps_c = constp.tile([128, 1], FP)
            nc.vector.memset(eps_c[:], EPS)
            onesrow_bf = constp.tile([1, 128], BF)
            nc.vector.memset(onesrow_bf[:], 1.0)

            # persistent
            qk_sb = [persp.tile([128, R], BF, name=f"qk{hc}") for hc in range(4)]
            alpha1 = persp.tile([128, 12], FP)
            beta1 = persp.tile([128, 12], FP)
            stats = persp.tile([128, 768], FP)
            statsg = persp.tile([128, 768], FP)
            gram_bf = persp.tile([128, 512], BF)
            meanx_bf = persp.tile([128, 2], BF)

            # W blocks for stats matmuls: (tiles, col0, width, flat-offset)
            wblocks = [(wqk_sb, 0, 512, 0),
                       (wv_sb, 0, 512, 512),
                       (wv_sb, 512, 512, 1024)]

            # ---------------- Phase A ----------------
            with tc.tile_pool(name="pA", bufs=2, space="PSUM") as pA, \
                 tc.tile_pool(name="sA", bufs=1) as sA:
                mean1s = sA.tile([128, 12], FP, tag="m1")
                vars_ = sA.tile([128, 12], FP, tag="vf")
                tmps = sA.tile([128, 12], FP, tag="tf")
                rstds = sA.tile([128, 12], FP, tag="rf")
                p_sb = [sA.tile([128, 1536], BF, tag=f"pp{c2}", name=f"p_sb{c2}")
                        for c2 in range(2)]
                with tc.tile_pool(name="xrp", bufs=1) as xrp, \
                     tc.tile_pool(name="pG", bufs=1, space="PSUM") as pG:
                    xr_sb = xrp.tile([128, 40 * C], BF)
                    for xh in (0, 1):
                        nc.sync.dma_start(
                            xr_sb[:, xh * 10 * C:(xh + 1) * 10 * C],
                            xr_d[:, xh * 10 * C:(xh + 1) * 10 * C])
                    for xh in (2, 3):
                        nc.scalar.dma_start(
                            xr_sb[:, xh * 10 * C:(xh + 1) * 10 * C],
                            xr_d[:, xh * 10 * C:(xh + 1) * 10 * C])
                    nc.vector.memset(stats[:, 512:768], 0.0)
                    for cc in range(2):
                        nc.sync.dma_start(xt_sb[cc][:],
                                          xt_d[cc * 128:(cc + 1) * 128, :])
                        nc.sync.dma_start(wqk_sb[cc][:],
                                          wqk_d[cc * 128:(cc + 1) * 128, :])
                        nc.sync.dma_start(wv_sb[cc][:],
                                          wv_d[cc * 128:(cc + 1) * 128, :])
                    nc.scalar.dma_start(wp_sb[:], wp_d[:])
                    nc.scalar.dma_start(eb_sb[:], eb_d[:])
                    nc.scalar.dma_start(g1c_sb[:], g1c_d[:])
                    nc.scalar.dma_start(b1c_sb[:], b1c_d[:])
                    nc.scalar.dma_start(id_sb[:], id_d[:])
                    gps = [pG.tile([128, C], FP, tag=f"g{cc}", name=f"gps{cc}")
                           for cc in range(2)]
                    csp = pG.tile([1, C], FP, tag="cs", name="csp")
                    rbord = ([rb for x in (0, 2) for rb in range(x * 10, x * 10 + 10)]
                             + [rb for x in (1, 3) for rb in range(x * 10, x * 10 + 10)])
                    for i, rb in enumerate(rbord):
                        fl = (i == 0)
                        ll = (i == 39)
                        nc.tensor.matmul(
                            gps[0][:],
                            xr_sb[:, rb * C: rb * C + 128],
                            xr_sb[:, rb * C:(rb + 1) * C],
                            start=fl, stop=ll)
                        nc.tensor.matmul(
                            gps[1][:, 128:256],
                            xr_sb[:, rb * C + 128: rb * C + 256],
                            xr_sb[:, rb * C + 128:(rb + 1) * C],
                            start=fl, stop=ll)
                        nc.tensor.matmul(
                            csp[:], ones_bf[:], xr_sb[:, rb * C:(rb + 1) * C],
                            start=fl, stop=ll)
                    nc.vector.tensor_copy(stats[:, 0:256], gps[0][:])
                    nc.vector.memset(stats[:, 256:384], 0.0)
                    nc.vector.tensor_copy(stats[:, 384:512], gps[1][:, 128:256])
                    nc.vector.tensor_copy(stats[0:1, 512:768], csp[:])
                bi = dramp.tile([128, 768], FP, tag="bi")
                bo = dramp.tile([128, 768], FP, tag="bo")
                nc.gpsimd.dma_start(bi[:], stats[:])
                nc.gpsimd.collective_compute(
                    "AllReduce", OP.add,
                    replica_groups=[list(range(NCORES))],
                    ins=[bi.opt()], outs=[bo.opt()])
                nc.gpsimd.dma_start(statsg[:], bo[:])

                # q/k matmuls (independent of stats; overlap the collective)
                for hc in range(4):
                    for rb in range(10):
                        pq = pA.tile([128, 512], FP, tag="pq")
                        for cc in range(2):
                            nc.tensor.matmul(
                                pq[:], wqk_sb[cc][:, hc * 128:(hc + 1) * 128],
                                xt_sb[cc][:, rb * 512:(rb + 1) * 512],
                                start=(cc == 0), stop=(cc == 1))
                        nc.scalar.copy(qk_sb[hc][:, rb * 512:(rb + 1) * 512],
                                       pq[:])

                # ---- BN1 stats math (gated on allreduce) ----
                with tc.tile_pool(name="pS1", bufs=1, space="PSUM") as pS1:
                    nc.gpsimd.tensor_copy(gram_bf[:, 0:128], statsg[:, 0:128])
                    nc.gpsimd.tensor_scalar(gram_bf[:, 128:256],
                                            statsg[:, 128:256], 2.0, None, OP.mult)
                    nc.gpsimd.tensor_copy(gram_bf[:, 384:512], statsg[:, 384:512])
                    for cc in range(2):
                        tpm = pS1.tile([128, 2], FP, tag="tp", bufs=1)
                        nc.tensor.transpose(
                            tpm[:, 0:1],
                            statsg[0:1, 512 + cc * 128:512 + (cc + 1) * 128],
                            id_sb[0:1, 0:1])
                        nc.vector.tensor_scalar(meanx_bf[:, cc:cc + 1], tpm[:, 0:1],
                                                1.0 / NT, None, OP.mult)
                    # mean1 [128,12] direct: out[:, hc] = sum_c W[c, hc-dims] mean_x[c]
                    m1p = pS1.tile([128, 12], FP, tag="m1p")
                    wcols = [(wqk_sb, 0), (wqk_sb, 128), (wqk_sb, 256), (wqk_sb, 384),
                             (wv_sb, 0), (wv_sb, 128), (wv_sb, 256), (wv_sb, 384),
                             (wv_sb, 512), (wv_sb, 640), (wv_sb, 768), (wv_sb, 896)]
                    for hc, (wt, c0) in enumerate(wcols):
                        for cc in range(2):
                            nc.tensor.matmul(m1p[:, hc:hc + 1],
                                             wt[cc][:, c0:c0 + 128],
                                             meanx_bf[:, cc:cc + 1],
                                             start=(cc == 0), stop=(cc == 1))
                    nc.vector.tensor_copy(mean1s[:], m1p[:])
                    # E2 decomposition using symmetric Gram:
                    # P0 = W0 .* (G00 W0); P1 = W1 .* (G11 W1 + 2 G01^T W0)
                    # where G01^T W0 is computed as M01[c2-chunk1, h] via
                    # stationary G01 [c1-part, c2-cols].
                    for (wt, c0, w, f0) in wblocks:
                        mm = pS1.tile([128, 512], FP, tag="mm", bufs=3)
                        nc.tensor.matmul(mm[:], gram_bf[:, 0:128],
                                         wt[0][:, c0:c0 + w])
                        nc.vector.tensor_tensor(p_sb[0][:, f0:f0 + w], mm[:],
                                                wt[0][:, c0:c0 + w], OP.mult)
                        mm2 = pS1.tile([128, 512], FP, tag="mm", bufs=3)
                        nc.tensor.matmul(mm2[:], gram_bf[:, 384:512],
                                         wt[1][:, c0:c0 + w], start=True, stop=False)
                        nc.tensor.matmul(mm2[:], gram_bf[:, 128:256],
                                         wt[0][:, c0:c0 + w], start=False, stop=True)
                        nc.vector.tensor_tensor(p_sb[1][:, f0:f0 + w], mm2[:],
                                                wt[1][:, c0:c0 + w], OP.mult)
                    # E2 [128,12] direct: out[:, hc] = sum_c2 P[c2, hc-dims]
                    e2p = pS1.tile([128, 12], FP, tag="e2p")
                    for hc in range(12):
                        for c2 in range(2):
                            nc.tensor.matmul(e2p[:, hc:hc + 1],
                                             p_sb[c2][:, hc * 128:(hc + 1) * 128],
                                             ones_bf[:],
                                             start=(c2 == 0), stop=(c2 == 1))
                    # var = E2/NT - mean1^2 + eps ; rstd = exp(-0.5 ln(var))
                    nc.vector.scalar_tensor_tensor(tmps[:], mean1s[:], -1.0,
                                                   mean1s[:], OP.mult, OP.mult)
                    nc.vector.scalar_tensor_tensor(vars_[:], e2p[:], 1.0 / NT,
                                                   tmps[:], OP.mult, OP.add)
                    nc.scalar.activation(tmps[:], vars_[:], AF.Ln, bias=eps_c[:])
                    nc.scalar.activation(rstds[:], tmps[:], AF.Exp, scale=-0.5)
                    nc.vector.tensor_tensor(alpha1[:], g1c_sb[:], rstds[:], OP.mult)
                    nc.vector.tensor_tensor(tmps[:], mean1s[:], alpha1[:], OP.mult)
                    nc.vector.tensor_tensor(beta1[:], b1c_sb[:], tmps[:], OP.subtract)

            if DBG_STOP == "A":
                for hc in range(4):
                    nc.vector.tensor_scalar(
                        qk_sb[hc][:], qk_sb[hc][:],
                        alpha1[:, hc:hc + 1], beta1[:, hc:hc + 1], OP.mult, OP.add)
                with tc.tile_pool(name="dbg", bufs=2) as dbgp:
                    for i, hc in enumerate((0, 2)):
                        for rb in range(10):
                            dq = dbgp.tile([128, 512], FP, tag="dq", name="dq")
                            nc.vector.tensor_copy(
                                dq[:], qk_sb[hc][:, rb * 512:(rb + 1) * 512])
                            nc.sync.dma_start(
                                y_d[i * 128:(i + 1) * 128, rb * 512:(rb + 1) * 512],
                                dq[:])
            else:
                # ---------- attention + output, pair-streamed ----------
                with tc.tile_pool(name="pSm", bufs=S_BUFS, space="PSUM") as pSm, \
                     tc.tile_pool(name="pAv", bufs=AV_BUFS, space="PSUM") as pAv, \
                     tc.tile_pool(name="pVm", bufs=VMM_BUFS, space="PSUM") as pVm, \
                     tc.tile_pool(name="pPj", bufs=PROJ_BUFS, space="PSUM") as pPj, \
                     tc.tile_pool(name="etp", bufs=ET_BUFS) as etp, \
                     tc.tile_pool(name="vtp", bufs=3) as vtp, \
                     tc.tile_pool(name="gpp", bufs=10) as gpp, \
                     tc.tile_pool(name="gpr", bufs=GP_BUFS) as gpr, \
                     tc.tile_pool(name="ysp", bufs=2) as ysp, \
                     tc.tile_pool(name="rrp", bufs=6) as rrp:
                    # v m-chunk windows in a pair: (name, row0-in-pair, width)
                    VCH = [("c0", 0, 128), ("c1", 128, 128),
                           ("c2", N, 128), ("c3", N + 128, 128)]
                    g_pairs = {}
                    for pair in range(NPAIR):
                        r0 = pair * PW
                        for hc in range(4):
                            nc.gpsimd.tensor_scalar(
                                qk_sb[hc][:, r0:r0 + PW], qk_sb[hc][:, r0:r0 + PW],
                                alpha1[:, hc:hc + 1], beta1[:, hc:hc + 1],
                                OP.mult, OP.add)
                        # v matmuls -> vT chunk tiles [m, 8*(128+1)] with alpha fold
                        vts = {}
                        for (cn, rr, wdt) in VCH:
                            vt = vtp.tile([128, NH * (DV + 1)], BF, tag=f"vt{cn}",
                                          name=f"vt{cn}")
                            vts[cn] = vt
                            vt3 = vt.rearrange("p (h x) -> p h x", x=DV + 1)
                            for dh in range(2):
                                vp = pVm.tile([128, 512], FP, tag="vp")
                                for cc in range(2):
                                    nc.tensor.matmul(
                                        vp[:], xt_sb[cc][:, r0 + rr:r0 + rr + wdt],
                                        wv_sb[cc][:, dh * 512:(dh + 1) * 512],
                                        start=(cc == 0), stop=(cc == 1))
                                nc.vector.tensor_copy(
                                    vt3[:, dh * 4:dh * 4 + 4, 0:128], vp[:])
                            nc.gpsimd.memset(vt[:, 128::129], 1.0)
                        vt = vtp.tile([128, NH * (DV + 1)], BF, tag="vt4", name="vt4")
                        vts["c4"] = vt
                        vt3 = vt.rearrange("p (h x) -> p h x", x=DV + 1)
                        for dh in range(2):
                            vp = pVm.tile([128, 512], FP, tag="vp")
                            for (bb, orow) in ((0, 0), (1, 64)):
                                for cc in range(2):
                                    nc.tensor.matmul(
                                        vp[orow:orow + 64, :],
                                        xt_sb[cc][:, r0 + bb * N + 256:
                                                 r0 + bb * N + 320],
                                        wv_sb[cc][:, dh * 512:(dh + 1) * 512],
                                        start=(cc == 0), stop=(cc == 1))
                            nc.vector.tensor_copy(
                                vt3[:, dh * 4:dh * 4 + 4, 0:128], vp[:])
                        nc.gpsimd.memset(vt[:, 128::129], 1.0)

                        g_pair = gpr.tile([128, NH * PW], BF, tag="gp", name="g_pair")
                        g_pairs[pair] = g_pair
                        for h in range(NH):
                            qc, qr = h // 4, 32 * (h % 4)
                            q0 = qk_sb[qc][qr:qr + 32, r0:r0 + N]
                            q1 = qk_sb[qc][qr:qr + 32, r0 + N:r0 + PW]
                            k0 = qk_sb[2 + qc][qr:qr + 32, r0:r0 + N]
                            k1 = qk_sb[2 + qc][qr:qr + 32, r0 + N:r0 + PW]
                            sdefs = [("s0", k0[:, 0:128], q0),
                                     ("s1", k0[:, 128:256], q0),
                                     ("s2", k1[:, 0:128], q1),
                                     ("s3", k1[:, 128:256], q1)]
                            ets = []
                            for (tg, kap, qap) in sdefs:
                                sp = pSm.tile([128, N], FP, tag="s", name="sp")
                                nc.tensor.matmul(sp[:], kap, qap,
                                                 tile_position=(qr, 0))
                                et = etp.tile([128, N], BF, tag=tg, name="et")
                                nc.scalar.activation(et[:], sp[:], AF.Exp, scale=SCALE)
                                ets.append(et)
                            sp = pSm.tile([128, N], FP, tag="s", name="sp4")
                            nc.tensor.matmul(sp[0:64, :], k0[:, 256:320], q0,
                                             tile_position=(qr, 0))
                            nc.tensor.matmul(sp[64:128, :], k1[:, 256:320], q1,
                                             tile_position=(qr, 64))
                            et4 = etp.tile([128, N], BF, tag="s4", name="et4")
                            nc.scalar.activation(et4[:], sp[:], AF.Exp, scale=SCALE)
                            ets.append(et4)
                            ebmap = [0, 1, 0, 1, 2]
                            for i, et in enumerate(ets):
                                ebs = eb_sb[:, (h * 3 + ebmap[i]) * N:
                                            (h * 3 + ebmap[i] + 1) * N]
                                if i < EBM_POOL:
                                    nc.gpsimd.tensor_tensor(et[:], et[:], ebs, OP.mult)
                                else:
                                    nc.vector.tensor_tensor(et[:], et[:], ebs, OP.mult)
                            avA = pAv.tile([128, 3 * 129], FP, tag="av", name="avA")
                            avB = pAv.tile([128, 3 * 129], FP, tag="av", name="avB")
                            regions = [(avA, 0, 0, 0), (avA, 129, 0, 1),
                                       (avB, 0, 1, 0), (avB, 129, 1, 1)]
                            vmap = [["c0", "c1", "c4"], ["c2", "c3", "c4"]]
                            emap = [[0, 1, 4], [2, 3, 4]]
                            for (dst, c0_, bb, ncx) in regions:
                                for mc in range(3):
                                    et = ets[emap[bb][mc]]
                                    vtt = vts[vmap[bb][mc]]
                                    if mc == 2:
                                        sr = bb * 64
                                        stp = et[sr:sr + 64, ncx * 128:ncx * 128 + 128]
                                        mvp = vtt[sr:sr + 64, h * 129:(h + 1) * 129]
                                    else:
                                        stp = et[:, ncx * 128:ncx * 128 + 128]
                                        mvp = vtt[:, h * 129:(h + 1) * 129]
                                    nc.tensor.matmul(dst[:, c0_:c0_ + 129], stp, mvp,
                                                     start=(mc == 0), stop=(mc == 2))
                            for bb in range(2):
                                orow = bb * 64
                                for mc in range(3):
                                    et = ets[emap[bb][mc]]
                                    vtt = vts[vmap[bb][mc]]
                                    if mc == 2:
                                        sr = bb * 64
                                        stp = et[sr:sr + 64, 256:320]
                                        mvp = vtt[sr:sr + 64, h * 129:(h + 1) * 129]
                                    else:
                                        stp = et[:, 256:320]
                                        mvp = vtt[:, h * 129:(h + 1) * 129]
                                    nc.tensor.matmul(
                                        avA[orow:orow + 64, 258:387], stp, mvp,
                                        start=(mc == 0), stop=(mc == 2))
                            rA = rrp.tile([128, 3], FP, tag="rA", name="rA")
                            rB = rrp.tile([128, 2], FP, tag="rB", name="rB")
                            nc.vector.reciprocal_approx_fast(rA[:], avA[:, 128::129])
                            nc.vector.reciprocal_approx_fast(rB[:],
                                                             avB[:, 128:300:129])
                            ddefs = [(avA, 0, rA, 0, 0), (avA, 129, rA, 1, 128),
                                     (avB, 0, rB, 0, N), (avB, 129, rB, 1, N + 128)]
                            for (src, c0_, rr_, ri, gc) in ddefs:
                                gp = gpp.tile([128, 128], BF, tag="gpre", name="gp")
                                nc.vector.tensor_scalar(
                                    gp[:], src[:, c0_:c0_ + 128], rr_[:, ri:ri + 1],
                                    None, OP.mult)
                                nc.sync.dma_start_transpose(
                                    g_pair[:, h * PW + gc:h * PW + gc + 128], gp[:])
                            gp = gpp.tile([128, 128], BF, tag="gpre", name="gp4")
                            nc.vector.tensor_scalar(
                                gp[:], avA[:, 258:386], rA[:, 2:3],
                                None, OP.mult)
                            nc.sync.dma_start_transpose(
                                g_pair[:, h * PW + 256:h * PW + 320], gp[0:64, :])
                            nc.sync.dma_start_transpose(
                                g_pair[:, h * PW + N + 256:h * PW + N + 320],
                                gp[64:128, :])
                        for h in range(NH):
                            nc.gpsimd.tensor_scalar(
                                g_pair[:, h * PW:(h + 1) * PW],
                                g_pair[:, h * PW:(h + 1) * PW],
                                alpha1[:, 4 + h:5 + h], beta1[:, 4 + h:5 + h],
                                OP.mult, OP.add)
                        flush = ((pair % 2 == 1 and pair < 6) or pair >= 6)
                        if flush:
                            plist = [pair] if pair >= 6 else [pair - 1, pair]
                            for p2 in plist:
                                if pair >= 6:
                                    hw_ = NH * PW // 2
                                    for gh in range(2):
                                        nc.scalar.activation(
                                            g_pairs[p2][:, gh * hw_:(gh + 1) * hw_],
                                            g_pairs[p2][:, gh * hw_:(gh + 1) * hw_],
                                            AF.Gelu)
                                else:
                                    nc.scalar.activation(g_pairs[p2][:],
                                                         g_pairs[p2][:], AF.Gelu)
                            for p2 in plist:
                                gp2 = g_pairs.pop(p2)
                                rr0 = p2 * PW
                                for cc in range(2):
                                    yst = ysp.tile([128, PW], FP, tag=f"y{cc}",
                                                   name="yst")
                                    for half in range(2):
                                        if p2 == NPAIR - 1 and (cc + half) % 2 == 1:
                                            pp = pAv.tile([128, 3 * 129], FP,
                                                          tag="av", name="ppav")
                                            pp = pp[:, 0:N]
                                        else:
                                            pp = pPj.tile([128, N], FP, tag="pp")
                                        for h in range(NH):
                                            nc.tensor.matmul(
                                                pp[:],
                                                wp_sb[:, h * C + cc * 128:
                                                      h * C + cc * 128 + 128],
                                                gp2[:, h * PW + half * N:
                                                    h * PW + (half + 1) * N],
                                                start=(h == 0), stop=(h == NH - 1))
                                        nc.vector.tensor_copy(
                                            yst[:, half * N:(half + 1) * N], pp[:])
                                    if p2 == NPAIR - 1:
                                        for yh in range(2):
                                            nc.sync.dma_start(
                                                y_d[cc * 128:(cc + 1) * 128,
                                                    rr0 + yh * N:rr0 + (yh + 1) * N],
                                                yst[:, yh * N:(yh + 1) * N])
                                    else:
                                        nc.sync.dma_start(
                                            y_d[cc * 128:(cc + 1) * 128,
                                                rr0:rr0 + PW],
                                            yst[:])

    nc.compile()
    return nc


_PROG = None


def _get_prog():
    global _PROG
    if _PROG is None:
        _PROG = build_program()
    return _PROG


def _host_prep(x, Wqkv, g1, b1, ab, Wproj, g2, b2, idxs):
    x = np.asarray(x, dtype=np.float32)
    Wqkv = np.asarray(Wqkv, dtype=np.float32)
    Wproj = np.asarray(Wproj, dtype=np.float32)
    g1 = np.asarray(g1, np.float32)
    b1 = np.asarray(b1, np.float32)
    idxs = np.asarray(idxs)
    qrows = np.concatenate([np.arange(h * 192, h * 192 + 32) for h in range(NH)])
    krows = np.concatenate([np.arange(h * 192 + 32, h * 192 + 64) for h in range(NH)])
    vrows = np.concatenate([np.arange(h * 192 + 64, h * 192 + 192) for h in range(NH)])
    wqk = np.ascontiguousarray(Wqkv[np.concatenate([qrows, krows]), :].T).astype(
        ml_dtypes.bfloat16)                                     # (256, 512)
    wv = np.ascontiguousarray(Wqkv[vrows, :].T).astype(ml_dtypes.bfloat16)
    wp = np.ascontiguousarray(
        Wproj.reshape(C, NH, DV).transpose(2, 1, 0).reshape(DV, NH * C)).astype(
        ml_dtypes.bfloat16)                                     # (128, 2048)
    perm = np.concatenate([qrows, krows, vrows])
    g1c = np.ascontiguousarray(g1[perm].reshape(12, 128).T)
    b1c = np.ascontiguousarray(b1[perm].reshape(12, 128).T)
    ebm = np.exp(np.asarray(ab, np.float32))[:, idxs]           # (8, 320, 320)
    ebp = np.zeros((128, NH * 3 * N), np.float32)
    for h in range(NH):
        ebp[:, (h * 3) * N:(h * 3 + 1) * N] = ebm[h, 0:128, :]
        ebp[:, (h * 3 + 1) * N:(h * 3 + 2) * N] = ebm[h, 128:256, :]
        ebp[0:64, (h * 3 + 2) * N:(h * 3 + 3) * N] = ebm[h, 256:320, :]
        ebp[64:128, (h * 3 + 2) * N:(h * 3 + 3) * N] = ebm[h, 256:320, :]
    common = {
        "wqk": wqk, "wv": wv, "wp": wp,
        "eb": ebp.astype(ml_dtypes.bfloat16),
        "g1c": g1c, "b1c": b1c,
        "ident": np.eye(128, dtype=np.float32),
    }
    in_maps = []
    for c in range(NCORES):
        xc = np.ascontiguousarray(x[c * BL:(c + 1) * BL].reshape(R, C))
        m = dict(common)
        m["xr"] = np.ascontiguousarray(
            xc.reshape(40, 128, C).transpose(1, 0, 2).reshape(128, 40 * C)).astype(
            ml_dtypes.bfloat16)
        m["xt"] = np.ascontiguousarray(xc.T).astype(ml_dtypes.bfloat16)
        in_maps.append(m)
    return in_maps


def _postprocess(res, g2, b2):
    ys = [np.asarray(res.results[c]["y"], np.float32) for c in range(NCORES)]
    z = np.concatenate(
        [y.T.reshape(BL, N, C) for y in ys], axis=0)            # (128, 320, 256)
    zf = z.reshape(-1, C).astype(np.float64)
    m = zf.mean(axis=0)
    v = zf.var(axis=0)
    out = (z - m.astype(np.float32)) * (1.0 / np.sqrt(v + EPS)).astype(np.float32) \
        * np.asarray(g2, np.float32) + np.asarray(b2, np.float32)
    return out.astype(np.float32)


def _run(in_maps, g2, b2, trace=False):
    nc = _get_prog()
    res = run_bass_kernel_spmd(nc, in_maps, core_ids=list(range(NCORES)),
                               trace=trace)
    return _postprocess(res, g2, b2), res


def kernel(**inputs):
    out, _ = _run(_host_prep(**inputs), inputs["g2"], inputs["b2"])
    return out


def run_raw(**inputs):
    """Return raw per-core y tensors (for debugging)."""
    nc = _get_prog()
    res = run_bass_kernel_spmd(nc, _host_prep(**inputs),
                               core_ids=list(range(NCORES)))
    return [np.asarray(res.results[c]["y"], np.float32) for c in range(NCORES)]


# revision 6
# speedup vs baseline: 1.0309x; 1.0054x over previous
"""Trainium2 Bass kernel v2 for nn_Attention (qkv+BN1 -> biased softmax attn -> gelu -> proj; BN2 on host).

Data-parallel over batch B=128 across 8 cores (16 batches = 8 "pairs" of 640 rows each).

Key structure (per core):
  - Host sends xT [256,5120] bf16 (moving operand for q/k/v matmuls) and
    x_rows packed [128, 40*256] bf16 (stationary for the x Gram matrix).
  - BN1 stats via Gram: var(qkv_h) = W_h (Sigma_x) W_h^T - mean1^2, with
    Sigma_x = sum_r x x^T allreduced EARLY (overlaps q/k/v matmuls).
  - Attention S[m-part, n-free] per (head, pair); the two 64-row mc2 chunks of
    a batch-pair share one PSUM tile (5 exps per (h,pair) instead of 6).
  - et = exp(scale*S) (Act) then *= exp(bias) gathered on host (DVE/Pool).
  - AV flipped: out[n, d] = sum_m et[m,n] v[m,d] with v stored [m, 8*(128+1)]
    (interleaved ones columns) so out col 128 = softmax denominator Z[n].
  - divide+BN1-v-beta: g_pre = (av * (1/Z)[n-part]) + beta_bc (Pool stt);
    alpha_v folded into the v PSUM->SBUF copy; then DMA-transpose to g_pair
    [d-part, row-free]; gelu per pair; proj -> y [c-part, row] fp32.
    BN2 (training-mode batchnorm, final linear op) done on HOST.
"""
import os
import numpy as np
import ml_dtypes

import concourse.bass as bass
import concourse.tile as tile
from concourse import bacc, mybir
from concourse.bass_utils import run_bass_kernel_spmd

NCORES = 8
TRACE_SIM = os.environ.get("KERN_TRACE_SIM", "") == "1"
DBG_STOP = os.environ.get("KERN_STOP", "")
B, N, C = 128, 320, 256
NH, DK, DV = 8, 32, 128
BL = B // NCORES             # 16 batches/core
R = BL * N                   # 5120 rows/core
NPAIR = BL // 2              # 8 pairs
PW = 2 * N                   # 640 rows/pair
NT = B * N                   # 40960 global rows
EPS = 1e-5
SCALE = DK ** -0.5
FP = mybir.dt.float32
BF = mybir.dt.bfloat16
AF = mybir.ActivationFunctionType
OP = mybir.AluOpType

S_BUFS = int(os.environ.get("K2_S_BUFS", "4"))
AV_BUFS = int(os.environ.get("K2_AV_BUFS", "2"))
VMM_BUFS = int(os.environ.get("K2_VMM_BUFS", "1"))
PROJ_BUFS = int(os.environ.get("K2_PROJ_BUFS", "1"))
ET_BUFS = int(os.environ.get("K2_ET_BUFS", "8"))
GP_BUFS = int(os.environ.get("K2_GP_BUFS", "3"))
EBM_POOL = int(os.environ.get("K2_EBM_POOL", "5"))  # of 5 eb-mults on Pool


def build_program():
    nc = bacc.Bacc("TRN2", target_bir_lowering=False, debug=False,
                   enable_asserts=False, num_devices=NCORES)
    xr_d = nc.dram_tensor("xr", [128, 40 * C], BF, kind="ExternalInput").ap()
    xt_d = nc.dram_tensor("xt", [C, R], BF, kind="ExternalInput").ap()
    wqk_d = nc.dram_tensor("wqk", [C, 2 * NH * DK], BF, kind="ExternalInput").ap()
    wv_d = nc.dram_tensor("wv", [C, NH * DV], BF, kind="ExternalInput").ap()
    wp_d = nc.dram_tensor("wp", [DV, NH * C], BF, kind="ExternalInput").ap()
    eb_d = nc.dram_tensor("eb", [128, NH * 3 * N], BF, kind="ExternalInput").ap()
    g1c_d = nc.dram_tensor("g1c", [128, 12], FP, kind="ExternalInput").ap()
    b1c_d = nc.dram_tensor("b1c", [128, 12], FP, kind="ExternalInput").ap()
    id_d = nc.dram_tensor("ident", [128, 128], FP, kind="ExternalInput").ap()
    y_d = nc.dram_tensor("y", [C, R], FP, kind="ExternalOutput").ap()

    with tile.TileContext(nc, trace_sim=TRACE_SIM) as tc:
        with tc.tile_pool(name="const", bufs=1) as constp, \
             tc.tile_pool(name="pers", bufs=1) as persp, \
             tc.tile_pool(name="dram", bufs=1, space="DRAM") as dramp:

            # ---------------- constants / inputs ----------------
            wqk_sb = [constp.tile([128, 2 * NH * DK], BF, name=f"wqk{cc}")
                      for cc in range(2)]
            wv_sb = [constp.tile([128, NH * DV], BF, name=f"wv{cc}")
                     for cc in range(2)]
            wp_sb = constp.tile([128, NH * C], BF)
            eb_sb = constp.tile([128, NH * 3 * N], BF)
            g1c_sb = constp.tile([128, 12], FP)
            b1c_sb = constp.tile([128, 12], FP)
            id_sb = constp.tile([128, 128], FP)
            xt_sb = [persp.tile([128, R], BF, name=f"xt{cc}") for cc in range(2)]
            ones_bf = constp.tile([128, 1], BF)
            nc.vector.memset(ones_bf[:], 1.0)
            eps_c = constp.tile([128, 1], FP)
            nc.vector.memset(eps_c[:], EPS)
            onesrow_bf = constp.tile([1, 128], BF)
            nc.vector.memset(onesrow_bf[:], 1.0)

            # persistent
            qk_sb = [persp.tile([128, R], BF, name=f"qk{hc}") for hc in range(4)]
            alpha1 = persp.tile([128, 12], FP)
            beta1 = persp.tile([128, 12], FP)
            stats = persp.tile([128, 768], FP)
            statsg = persp.tile([128, 768], FP)
            gram_bf = persp.tile([128, 512], BF)
            meanx_bf = persp.tile([128, 2], BF)

            # W blocks for stats matmuls: (tiles, col0, width, flat-offset)
            wblocks = [(wqk_sb, 0, 512, 0),
                       (wv_sb, 0, 512, 512),
                       (wv_sb, 512, 512, 1024)]

            # ---------------- Phase A ----------------
            with tc.tile_pool(name="pA", bufs=2, space="PSUM") as pA, \
                 tc.tile_pool(name="sA", bufs=1) as sA:
                mean1s = sA.tile([128, 12], FP, tag="m1")
                vars_ = sA.tile([128, 12], FP, tag="vf")
                tmps = sA.tile([128, 12], FP, tag="tf")
                rstds = sA.tile([128, 12], FP, tag="rf")
                p_sb = [sA.tile([128, 1536], BF, tag=f"pp{c2}", name=f"p_sb{c2}")
                        for c2 in range(2)]
                with tc.tile_pool(name="xrp", bufs=1) as xrp, \
                     tc.tile_pool(name="pG", bufs=1, space="PSUM") as pG:
                    xr_sb = xrp.tile([128, 40 * C], BF)
                    for xh in (0, 1):
                        nc.sync.dma_start(
                            xr_sb[:, xh * 10 * C:(xh + 1) * 10 * C],
                            xr_d[:, xh * 10 * C:(xh + 1) * 10 * C])
                    for xh in (2, 3):
                        nc.scalar.dma_start(
                            xr_sb[:, xh * 10 * C:(xh + 1) * 10 * C],
                            xr_d[:, xh * 10 * C:(xh + 1) * 10 * C])
                    nc.vector.memset(stats[:, 512:768], 0.0)
                    for cc in range(2):
                        nc.sync.dma_start(xt_sb[cc][:],
                                          xt_d[cc * 128:(cc + 1) * 128, :])
                        nc.sync.dma_start(wqk_sb[cc][:],
                                          wqk_d[cc * 128:(cc + 1) * 128, :])
                        nc.sync.dma_start(wv_sb[cc][:],
                                          wv_d[cc * 128:(cc + 1) * 128, :])
                    nc.scalar.dma_start(wp_sb[:], wp_d[:])
                    nc.scalar.dma_start(eb_sb[:], eb_d[:])
                    nc.scalar.dma_start(g1c_sb[:], g1c_d[:])
                    nc.scalar.dma_start(b1c_sb[:], b1c_d[:])
                    nc.scalar.dma_start(id_sb[:], id_d[:])
                    gps = [pG.tile([128, C], FP, tag=f"g{cc}", name=f"gps{cc}")
                           for cc in range(2)]
                    csp = pG.tile([1, C], FP, tag="cs", name="csp")
                    # PE warm-up: ramp the tensor engine to full p-state while
                    # the xr DMA is in flight so the Gram runs at full clock.
                    wrm = sA.tile([128, 128], BF, tag="wrm")
                    nc.vector.memset(wrm[:], 0.0)
                    wps = pG.tile([128, 128], FP, tag="wp", name="wps")
                    for _ in range(20):
                        nc.tensor.matmul(wps[:], wrm[:], wrm[:])
                    rbord = ([rb for x in (0, 2) for rb in range(x * 10, x * 10 + 10)]
                             + [rb for x in (1, 3) for rb in range(x * 10, x * 10 + 10)])
                    for i, rb in enumerate(rbord):
                        fl = (i == 0)
                        ll = (i == 39)
                        nc.tensor.matmul(
                            gps[0][:],
                            xr_sb[:, rb * C: rb * C + 128],
                            xr_sb[:, rb * C:(rb + 1) * C],
                            start=fl, stop=ll)
                        nc.tensor.matmul(
                            gps[1][:, 128:256],
                            xr_sb[:, rb * C + 128: rb * C + 256],
                            xr_sb[:, rb * C + 128:(rb + 1) * C],
                            start=fl, stop=ll)
                        nc.tensor.matmul(
                            csp[:], ones_bf[:], xr_sb[:, rb * C:(rb + 1) * C],
                            start=fl, stop=ll)
                    nc.vector.tensor_copy(stats[:, 0:256], gps[0][:])
                    nc.vector.memset(stats[:, 256:384], 0.0)
                    nc.vector.tensor_copy(stats[:, 384:512], gps[1][:, 128:256])
                    nc.vector.tensor_copy(stats[0:1, 512:768], csp[:])
                bi = dramp.tile([128, 768], FP, tag="bi")
                bo = dramp.tile([128, 768], FP, tag="bo")
                nc.gpsimd.dma_start(bi[:], stats[:])
                nc.gpsimd.collective_compute(
                    "AllReduce", OP.add,
                    replica_groups=[list(range(NCORES))],
                    ins=[bi.opt()], outs=[bo.opt()])
                nc.gpsimd.dma_start(statsg[:], bo[:])

                # q/k matmuls (independent of stats; overlap the collective)
                for hc in range(4):
                    for rb in range(10):
                        pq = pA.tile([128, 512], FP, tag="pq")
                        for cc in range(2):
                            nc.tensor.matmul(
                                pq[:], wqk_sb[cc][:, hc * 128:(hc + 1) * 128],
                                xt_sb[cc][:, rb * 512:(rb + 1) * 512],
                                start=(cc == 0), stop=(cc == 1))
                        nc.scalar.copy(qk_sb[hc][:, rb * 512:(rb + 1) * 512],
                                       pq[:])

                # ---- BN1 stats math (gated on allreduce) ----
                with tc.tile_pool(name="pS1", bufs=1, space="PSUM") as pS1:
                    nc.gpsimd.tensor_copy(gram_bf[:, 0:128], statsg[:, 0:128])
                    nc.gpsimd.tensor_scalar(gram_bf[:, 128:256],
                                            statsg[:, 128:256], 2.0, None, OP.mult)
                    nc.gpsimd.tensor_copy(gram_bf[:, 384:512], statsg[:, 384:512])
                    for cc in range(2):
                        tpm = pS1.tile([128, 2], FP, tag="tp", bufs=1)
                        nc.tensor.transpose(
                            tpm[:, 0:1],
                            statsg[0:1, 512 + cc * 128:512 + (cc + 1) * 128],
                            id_sb[0:1, 0:1])
                        nc.vector.tensor_scalar(meanx_bf[:, cc:cc + 1], tpm[:, 0:1],
                                                1.0 / NT, None, OP.mult)
                    # mean1 [128,12] direct: out[:, hc] = sum_c W[c, hc-dims] mean_x[c]
                    m1p = pS1.tile([128, 12], FP, tag="m1p")
                    wcols = [(wqk_sb, 0), (wqk_sb, 128), (wqk_sb, 256), (wqk_sb, 384),
                             (wv_sb, 0), (wv_sb, 128), (wv_sb, 256), (wv_sb, 384),
                             (wv_sb, 512), (wv_sb, 640), (wv_sb, 768), (wv_sb, 896)]
                    for hc, (wt, c0) in enumerate(wcols):
                        for cc in range(2):
                            nc.tensor.matmul(m1p[:, hc:hc + 1],
                                             wt[cc][:, c0:c0 + 128],
                                             meanx_bf[:, cc:cc + 1],
                                             start=(cc == 0), stop=(cc == 1))
                    nc.vector.tensor_copy(mean1s[:], m1p[:])
                    # E2 decomposition using symmetric Gram:
                    # P0 = W0 .* (G00 W0); P1 = W1 .* (G11 W1 + 2 G01^T W0)
                    # where G01^T W0 is computed as M01[c2-chunk1, h] via
                    # stationary G01 [c1-part, c2-cols].
                    for (wt, c0, w, f0) in wblocks:
                        mm = pS1.tile([128, 512], FP, tag="mm", bufs=3)
                        nc.tensor.matmul(mm[:], gram_bf[:, 0:128],
                                         wt[0][:, c0:c0 + w])
                        nc.vector.tensor_tensor(p_sb[0][:, f0:f0 + w], mm[:],
                                                wt[0][:, c0:c0 + w], OP.mult)
                        mm2 = pS1.tile([128, 512], FP, tag="mm", bufs=3)
                        nc.tensor.matmul(mm2[:], gram_bf[:, 384:512],
                                         wt[1][:, c0:c0 + w], start=True, stop=False)
                        nc.tensor.matmul(mm2[:], gram_bf[:, 128:256],
                                         wt[0][:, c0:c0 + w], start=False, stop=True)
                        nc.vector.tensor_tensor(p_sb[1][:, f0:f0 + w], mm2[:],
                                                wt[1][:, c0:c0 + w], OP.mult)
                    # E2 [128,12] direct: out[:, hc] = sum_c2 P[c2, hc-dims]
                    e2p = pS1.tile([128, 12], FP, tag="e2p")
                    for hc in range(12):
                        for c2 in range(2):
                            nc.tensor.matmul(e2p[:, hc:hc + 1],
                                             p_sb[c2][:, hc * 128:(hc + 1) * 128],
                                             ones_bf[:],
                                             start=(c2 == 0), stop=(c2 == 1))
                    # var = E2/NT - mean1^2 + eps ; rstd = exp(-0.5 ln(var))
                    nc.vector.scalar_tensor_tensor(tmps[:], mean1s[:], -1.0,
                                                   mean1s[:], OP.mult, OP.mult)
                    nc.vector.scalar_tensor_tensor(vars_[:], e2p[:], 1.0 / NT,
                                                   tmps[:], OP.mult, OP.add)
                    nc.scalar.activation(tmps[:], vars_[:], AF.Ln, bias=eps_c[:])
                    nc.scalar.activation(rstds[:], tmps[:], AF.Exp, scale=-0.5)
                    nc.vector.tensor_tensor(alpha1[:], g1c_sb[:], rstds[:], OP.mult)
                    nc.vector.tensor_tensor(tmps[:], mean1s[:], alpha1[:], OP.mult)
                    nc.vector.tensor_tensor(beta1[:], b1c_sb[:], tmps[:], OP.subtract)

            if DBG_STOP == "A":
                for hc in range(4):
                    nc.vector.tensor_scalar(
                        qk_sb[hc][:], qk_sb[hc][:],
                        alpha1[:, hc:hc + 1], beta1[:, hc:hc + 1], OP.mult, OP.add)
                with tc.tile_pool(name="dbg", bufs=2) as dbgp:
                    for i, hc in enumerate((0, 2)):
                        for rb in range(10):
                            dq = dbgp.tile([128, 512], FP, tag="dq", name="dq")
                            nc.vector.tensor_copy(
                                dq[:], qk_sb[hc][:, rb * 512:(rb + 1) * 512])
                            nc.sync.dma_start(
                                y_d[i * 128:(i + 1) * 128, rb * 512:(rb + 1) * 512],
                                dq[:])
            else:
                # ---------- attention + output, pair-streamed ----------
                with tc.tile_pool(name="pSm", bufs=S_BUFS, space="PSUM") as pSm, \
                     tc.tile_pool(name="pAv", bufs=AV_BUFS, space="PSUM") as pAv, \
                     tc.tile_pool(name="pVm", bufs=VMM_BUFS, space="PSUM") as pVm, \
                     tc.tile_pool(name="pPj", bufs=PROJ_BUFS, space="PSUM") as pPj, \
                     tc.tile_pool(name="etp", bufs=ET_BUFS) as etp, \
                     tc.tile_pool(name="vtp", bufs=3) as vtp, \
                     tc.tile_pool(name="gpp", bufs=10) as gpp, \
                     tc.tile_pool(name="gpr", bufs=GP_BUFS) as gpr, \
                     tc.tile_pool(name="ysp", bufs=2) as ysp, \
                     tc.tile_pool(name="rrp", bufs=6) as rrp:
                    # v m-chunk windows in a pair: (name, row0-in-pair, width)
                    VCH = [("c0", 0, 128), ("c1", 128, 128),
                           ("c2", N, 128), ("c3", N + 128, 128)]
                    g_pairs = {}
                    for pair in range(NPAIR):
                        r0 = pair * PW
                        for hc in range(4):
                            nc.gpsimd.tensor_scalar(
                                qk_sb[hc][:, r0:r0 + PW], qk_sb[hc][:, r0:r0 + PW],
                                alpha1[:, hc:hc + 1], beta1[:, hc:hc + 1],
                                OP.mult, OP.add)
                        # v matmuls -> vT chunk tiles [m, 8*(128+1)] with alpha fold
                        vts = {}
                        for (cn, rr, wdt) in VCH:
                            vt = vtp.tile([128, NH * (DV + 1)], BF, tag=f"vt{cn}",
                                          name=f"vt{cn}")
                            vts[cn] = vt
                            vt3 = vt.rearrange("p (h x) -> p h x", x=DV + 1)
                            for dh in range(2):
                                vp = pVm.tile([128, 512], FP, tag="vp")
                                for cc in range(2):
                                    nc.tensor.matmul(
                                        vp[:], xt_sb[cc][:, r0 + rr:r0 + rr + wdt],
                                        wv_sb[cc][:, dh * 512:(dh + 1) * 512],
                                        start=(cc == 0), stop=(cc == 1))
                                nc.vector.tensor_copy(
                                    vt3[:, dh * 4:dh * 4 + 4, 0:128], vp[:])
                            nc.gpsimd.memset(vt[:, 128::129], 1.0)
                        vt = vtp.tile([128, NH * (DV + 1)], BF, tag="vt4", name="vt4")
                        vts["c4"] = vt
                        vt3 = vt.rearrange("p (h x) -> p h x", x=DV + 1)
                        for dh in range(2):
                            vp = pVm.tile([128, 512], FP, tag="vp")
                            for (bb, orow) in ((0, 0), (1, 64)):
                                for cc in range(2):
                                    nc.tensor.matmul(
                                        vp[orow:orow + 64, :],
                                        xt_sb[cc][:, r0 + bb * N + 256:
                                                 r0 + bb * N + 320],
                                        wv_sb[cc][:, dh * 512:(dh + 1) * 512],
                                        start=(cc == 0), stop=(cc == 1))
                            nc.vector.tensor_copy(
                                vt3[:, dh * 4:dh * 4 + 4, 0:128], vp[:])
                        nc.gpsimd.memset(vt[:, 128::129], 1.0)

                        g_pair = gpr.tile([128, NH * PW], BF, tag="gp", name="g_pair")
                        g_pairs[pair] = g_pair
                        for h in range(NH):
                            qc, qr = h // 4, 32 * (h % 4)
                            q0 = qk_sb[qc][qr:qr + 32, r0:r0 + N]
                            q1 = qk_sb[qc][qr:qr + 32, r0 + N:r0 + PW]
                            k0 = qk_sb[2 + qc][qr:qr + 32, r0:r0 + N]
                            k1 = qk_sb[2 + qc][qr:qr + 32, r0 + N:r0 + PW]
                            sdefs = [("s0", k0[:, 0:128], q0),
                                     ("s1", k0[:, 128:256], q0),
                                     ("s2", k1[:, 0:128], q1),
                                     ("s3", k1[:, 128:256], q1)]
                            ets = []
                            for (tg, kap, qap) in sdefs:
                                sp = pSm.tile([128, N], FP, tag="s", name="sp")
                                nc.tensor.matmul(sp[:], kap, qap,
                                                 tile_position=(qr, 0))
                                et = etp.tile([128, N], BF, tag=tg, name="et")
                                nc.scalar.activation(et[:], sp[:], AF.Exp, scale=SCALE)
                                ets.append(et)
                            sp = pSm.tile([128, N], FP, tag="s", name="sp4")
                            nc.tensor.matmul(sp[0:64, :], k0[:, 256:320], q0,
                                             tile_position=(qr, 0))
                            nc.tensor.matmul(sp[64:128, :], k1[:, 256:320], q1,
                                             tile_position=(qr, 64))
                            et4 = etp.tile([128, N], BF, tag="s4", name="et4")
                            nc.scalar.activation(et4[:], sp[:], AF.Exp, scale=SCALE)
                            ets.append(et4)
                            ebmap = [0, 1, 0, 1, 2]
                            for i, et in enumerate(ets):
                                ebs = eb_sb[:, (h * 3 + ebmap[i]) * N:
                                            (h * 3 + ebmap[i] + 1) * N]
                                if i < EBM_POOL:
                                    nc.gpsimd.tensor_tensor(et[:], et[:], ebs, OP.mult)
                                else:
                                    nc.vector.tensor_tensor(et[:], et[:], ebs, OP.mult)
                            avA = pAv.tile([128, 3 * 129], FP, tag="av", name="avA")
                            avB = pAv.tile([128, 3 * 129], FP, tag="av", name="avB")
                            regions = [(avA, 0, 0, 0), (avA, 129, 0, 1),
                                       (avB, 0, 1, 0), (avB, 129, 1, 1)]
                            vmap = [["c0", "c1", "c4"], ["c2", "c3", "c4"]]
                            emap = [[0, 1, 4], [2, 3, 4]]
                            for (dst, c0_, bb, ncx) in regions:
                                for mc in range(3):
                                    et = ets[emap[bb][mc]]
                                    vtt = vts[vmap[bb][mc]]
                                    if mc == 2:
                                        sr = bb * 64
                                        stp = et[sr:sr + 64, ncx * 128:ncx * 128 + 128]
                                        mvp = vtt[sr:sr + 64, h * 129:(h + 1) * 129]
                                    else:
                                        stp = et[:, ncx * 128:ncx * 128 + 128]
                                        mvp = vtt[:, h * 129:(h + 1) * 129]
                                    nc.tensor.matmul(dst[:, c0_:c0_ + 129], stp, mvp,
                                                     start=(mc == 0), stop=(mc == 2))
                            for bb in range(2):
                                orow = bb * 64
                                for mc in range(3):
                                    et = ets[emap[bb][mc]]
                                    vtt = vts[vmap[bb][mc]]
                                    if mc == 2:
                                        sr = bb * 64
                                        stp = et[sr:sr + 64, 256:320]
                                        mvp = vtt[sr:sr + 64, h * 129:(h + 1) * 129]
                                    else:
                                        stp = et[:, 256:320]
                                        mvp = vtt[:, h * 129:(h + 1) * 129]
                                    nc.tensor.matmul(
                                        avA[orow:orow + 64, 258:387], stp, mvp,
                                        start=(mc == 0), stop=(mc == 2))
                            rA = rrp.tile([128, 3], FP, tag="rA", name="rA")
                            rB = rrp.tile([128, 2], FP, tag="rB", name="rB")
                            nc.vector.reciprocal_approx_fast(rA[:], avA[:, 128::129])
                            nc.vector.reciprocal_approx_fast(rB[:],
                                                             avB[:, 128:300:129])
                            ddefs = [(avA, 0, rA, 0, 0), (avA, 129, rA, 1, 128),
                                     (avB, 0, rB, 0, N), (avB, 129, rB, 1, N + 128)]
                            for (src, c0_, rr_, ri, gc) in ddefs:
                                gp = gpp.tile([128, 128], BF, tag="gpre", name="gp")
                                nc.vector.tensor_scalar(
                                    gp[:], src[:, c0_:c0_ + 128], rr_[:, ri:ri + 1],
                                    None, OP.mult)
                                nc.sync.dma_start_transpose(
                                    g_pair[:, h * PW + gc:h * PW + gc + 128], gp[:])
                            gp = gpp.tile([128, 128], BF, tag="gpre", name="gp4")
                            nc.vector.tensor_scalar(
                                gp[:], avA[:, 258:386], rA[:, 2:3],
                                None, OP.mult)
                            nc.sync.dma_start_transpose(
                                g_pair[:, h * PW + 256:h * PW + 320], gp[0:64, :])
                            nc.sync.dma_start_transpose(
                                g_pair[:, h * PW + N + 256:h * PW + N + 320],
                                gp[64:128, :])
                        for h in range(NH):
                            nc.gpsimd.tensor_scalar(
                                g_pair[:, h * PW:(h + 1) * PW],
                                g_pair[:, h * PW:(h + 1) * PW],
                                alpha1[:, 4 + h:5 + h], beta1[:, 4 + h:5 + h],
                                OP.mult, OP.add)
                        flush = ((pair % 2 == 1 and pair < 6) or pair >= 6)
                        if flush:
                            plist = [pair] if pair >= 6 else [pair - 1, pair]
                            for p2 in plist:
                                if pair >= 6:
                                    hw_ = NH * PW // 2
                                    for gh in range(2):
                                        nc.scalar.activation(
                                            g_pairs[p2][:, gh * hw_:(gh + 1) * hw_],
                                            g_pairs[p2][:, gh * hw_:(gh + 1) * hw_],
                                            AF.Gelu)
                                else:
                                    nc.scalar.activation(g_pairs[p2][:],
                                                         g_pairs[p2][:], AF.Gelu)
                            for p2 in plist:
                                gp2 = g_pairs.pop(p2)
                                rr0 = p2 * PW
                                for cc in range(2):
                                    yst = ysp.tile([128, PW], FP, tag=f"y{cc}",
                                                   name="yst")
                                    for half in range(2):
                                        if p2 == NPAIR - 1 and (cc + half) % 2 == 1:
                                            pp = pAv.tile([128, 3 * 129], FP,
                                                          tag="av", name="ppav")
                                            pp = pp[:, 0:N]
                                        else:
                                            pp = pPj.tile([128, N], FP, tag="pp")
                                        for h in range(NH):
                                            nc.tensor.matmul(
                                                pp[:],
                                                wp_sb[:, h * C + cc * 128:
                                                      h * C + cc * 128 + 128],
                                                gp2[:, h * PW + half * N:
                                                    h * PW + (half + 1) * N],
                                                start=(h == 0), stop=(h == NH - 1))
                                        nc.vector.tensor_copy(
                                            yst[:, half * N:(half + 1) * N], pp[:])
                                    if p2 == NPAIR - 1:
                                        for yh in range(2):
                                            nc.sync.dma_start(
                                                y_d[cc * 128:(cc + 1) * 128,
                                                    rr0 + yh * N:rr0 + (yh + 1) * N],
                                                yst[:, yh * N:(yh + 1) * N])
                                    else:
                                        nc.sync.dma_start(
                                            y_d[cc * 128:(cc + 1) * 128,
                                                rr0:rr0 + PW],
                                            yst[:])

    nc.compile()
    return nc


_PROG = None


def _get_prog():
    global _PROG
    if _PROG is None:
        _PROG = build_program()
    return _PROG


def _host_prep(x, Wqkv, g1, b1, ab, Wproj, g2, b2, idxs):
    x = np.asarray(x, dtype=np.float32)
    Wqkv = np.asarray(Wqkv, dtype=np.float32)
    Wproj = np.asarray(Wproj, dtype=np.float32)
    g1 = np.asarray(g1, np.float32)
    b1 = np.asarray(b1, np.float32)
    idxs = np.asarray(idxs)
    qrows = np.concatenate([np.arange(h * 192, h * 192 + 32) for h in range(NH)])
    krows = np.concatenate([np.arange(h * 192 + 32, h * 192 + 64) for h in range(NH)])
    vrows = np.concatenate([np.arange(h * 192 + 64, h * 192 + 192) for h in range(NH)])
    wqk = np.ascontiguousarray(Wqkv[np.concatenate([qrows, krows]), :].T).astype(
        ml_dtypes.bfloat16)                                     # (256, 512)
    wv = np.ascontiguousarray(Wqkv[vrows, :].T).astype(ml_dtypes.bfloat16)
    wp = np.ascontiguousarray(
        Wproj.reshape(C, NH, DV).transpose(2, 1, 0).reshape(DV, NH * C)).astype(
        ml_dtypes.bfloat16)                                     # (128, 2048)
    perm = np.concatenate([qrows, krows, vrows])
    g1c = np.ascontiguousarray(g1[perm].reshape(12, 128).T)
    b1c = np.ascontiguousarray(b1[perm].reshape(12, 128).T)
    ebm = np.exp(np.asarray(ab, np.float32))[:, idxs]           # (8, 320, 320)
    ebp = np.zeros((128, NH * 3 * N), np.float32)
    for h in range(NH):
        ebp[:, (h * 3) * N:(h * 3 + 1) * N] = ebm[h, 0:128, :]
        ebp[:, (h * 3 + 1) * N:(h * 3 + 2) * N] = ebm[h, 128:256, :]
        ebp[0:64, (h * 3 + 2) * N:(h * 3 + 3) * N] = ebm[h, 256:320, :]
        ebp[64:128, (h * 3 + 2) * N:(h * 3 + 3) * N] = ebm[h, 256:320, :]
    common = {
        "wqk": wqk, "wv": wv, "wp": wp,
        "eb": ebp.astype(ml_dtypes.bfloat16),
        "g1c": g1c, "b1c": b1c,
        "ident": np.eye(128, dtype=np.float32),
    }
    in_maps = []
    for c in range(NCORES):
        xc = np.ascontiguousarray(x[c * BL:(c + 1) * BL].reshape(R, C))
        m = dict(common)
        m["xr"] = np.ascontiguousarray(
            xc.reshape(40, 128, C).transpose(1, 0, 2).reshape(128, 40 * C)).astype(
            ml_dtypes.bfloat16)
        m["xt"] = np.ascontiguousarray(xc.T).astype(ml_dtypes.bfloat16)
        in_maps.append(m)
    return in_maps


def _postprocess(res, g2, b2):
    ys = [np.asarray(res.results[c]["y"], np.float32) for c in range(NCORES)]
    z = np.concatenate(
        [y.T.reshape(BL, N, C) for y in ys], axis=0)            # (128, 320, 256)
    zf = z.reshape(-1, C).astype(np.float64)
    m = zf.mean(axis=0)
    v = zf.var(axis=0)
    out = (z - m.astype(np.float32)) * (1.0 / np.sqrt(v + EPS)).astype(np.float32) \
        * np.asarray(g2, np.float32) + np.asarray(b2, np.float32)
    return out.astype(np.float32)


def _run(in_maps, g2, b2, trace=False):
    nc = _get_prog()
    res = run_bass_kernel_spmd(nc, in_maps, core_ids=list(range(NCORES)),
                               trace=trace)
    return _postprocess(res, g2, b2), res


def kernel(**inputs):
    out, _ = _run(_host_prep(**inputs), inputs["g2"], inputs["b2"])
    return out


def run_raw(**inputs):
    """Return raw per-core y tensors (for debugging)."""
    nc = _get_prog()
    res = run_bass_kernel_spmd(nc, _host_prep(**inputs),
                               core_ids=list(range(NCORES)))
    return [np.asarray(res.results[c]["y"], np.float32) for c in range(NCORES)]


# revision 7
# speedup vs baseline: 1.0311x; 1.0002x over previous
"""Trainium2 Bass kernel v2 for nn_Attention (qkv+BN1 -> biased softmax attn -> gelu -> proj; BN2 on host).

Data-parallel over batch B=128 across 8 cores (16 batches = 8 "pairs" of 640 rows each).

Key structure (per core):
  - Host sends xT [256,5120] bf16 (moving operand for q/k/v matmuls) and
    x_rows packed [128, 40*256] bf16 (stationary for the x Gram matrix).
  - BN1 stats via Gram: var(qkv_h) = W_h (Sigma_x) W_h^T - mean1^2, with
    Sigma_x = sum_r x x^T allreduced EARLY (overlaps q/k/v matmuls).
  - Attention S[m-part, n-free] per (head, pair); the two 64-row mc2 chunks of
    a batch-pair share one PSUM tile (5 exps per (h,pair) instead of 6).
  - et = exp(scale*S) (Act) then *= exp(bias) gathered on host (DVE/Pool).
  - AV flipped: out[n, d] = sum_m et[m,n] v[m,d] with v stored [m, 8*(128+1)]
    (interleaved ones columns) so out col 128 = softmax denominator Z[n].
  - divide+BN1-v-beta: g_pre = (av * (1/Z)[n-part]) + beta_bc (Pool stt);
    alpha_v folded into the v PSUM->SBUF copy; then DMA-transpose to g_pair
    [d-part, row-free]; gelu per pair; proj -> y [c-part, row] fp32.
    BN2 (training-mode batchnorm, final linear op) done on HOST.
"""
import os
import numpy as np
import ml_dtypes

import concourse.bass as bass
import concourse.tile as tile
from concourse import bacc, mybir
from concourse.bass_utils import run_bass_kernel_spmd

NCORES = 8
TRACE_SIM = os.environ.get("KERN_TRACE_SIM", "") == "1"
DBG_STOP = os.environ.get("KERN_STOP", "")
B, N, C = 128, 320, 256
NH, DK, DV = 8, 32, 128
BL = B // NCORES             # 16 batches/core
R = BL * N                   # 5120 rows/core
NPAIR = BL // 2              # 8 pairs
PW = 2 * N                   # 640 rows/pair
NT = B * N                   # 40960 global rows
EPS = 1e-5
SCALE = DK ** -0.5
FP = mybir.dt.float32
BF = mybir.dt.bfloat16
AF = mybir.ActivationFunctionType
OP = mybir.AluOpType

S_BUFS = int(os.environ.get("K2_S_BUFS", "4"))
AV_BUFS = int(os.environ.get("K2_AV_BUFS", "2"))
VMM_BUFS = int(os.environ.get("K2_VMM_BUFS", "1"))
PROJ_BUFS = int(os.environ.get("K2_PROJ_BUFS", "1"))
ET_BUFS = int(os.environ.get("K2_ET_BUFS", "8"))
GP_BUFS = int(os.environ.get("K2_GP_BUFS", "3"))
EBM_POOL = int(os.environ.get("K2_EBM_POOL", "5"))  # of 5 eb-mults on Pool


def build_program():
    nc = bacc.Bacc("TRN2", target_bir_lowering=False, debug=False,
                   enable_asserts=False, num_devices=NCORES)
    xr_d = nc.dram_tensor("xr", [128, 40 * C], BF, kind="ExternalInput").ap()
    xt_d = nc.dram_tensor("xt", [C, R], BF, kind="ExternalInput").ap()
    wqk_d = nc.dram_tensor("wqk", [C, 2 * NH * DK], BF, kind="ExternalInput").ap()
    wv_d = nc.dram_tensor("wv", [C, NH * DV], BF, kind="ExternalInput").ap()
    wp_d = nc.dram_tensor("wp", [DV, NH * C], BF, kind="ExternalInput").ap()
    eb_d = nc.dram_tensor("eb", [128, NH * 3 * N], BF, kind="ExternalInput").ap()
    g1c_d = nc.dram_tensor("g1c", [128, 12], FP, kind="ExternalInput").ap()
    b1c_d = nc.dram_tensor("b1c", [128, 12], FP, kind="ExternalInput").ap()
    id_d = nc.dram_tensor("ident", [128, 128], FP, kind="ExternalInput").ap()
    y_d = nc.dram_tensor("y", [C, R], FP, kind="ExternalOutput").ap()

    with tile.TileContext(nc, trace_sim=TRACE_SIM) as tc:
        with tc.tile_pool(name="const", bufs=1) as constp, \
             tc.tile_pool(name="pers", bufs=1) as persp, \
             tc.tile_pool(name="dram", bufs=1, space="DRAM") as dramp:

            # ---------------- constants / inputs ----------------
            wqk_sb = [constp.tile([128, 2 * NH * DK], BF, name=f"wqk{cc}")
                      for cc in range(2)]
            wv_sb = [constp.tile([128, NH * DV], BF, name=f"wv{cc}")
                     for cc in range(2)]
            wp_sb = constp.tile([128, NH * C], BF)
            eb_sb = constp.tile([128, NH * 3 * N], BF)
            g1c_sb = constp.tile([128, 12], FP)
            b1c_sb = constp.tile([128, 12], FP)
            id_sb = constp.tile([128, 128], FP)
            xt_sb = [persp.tile([128, R], BF, name=f"xt{cc}") for cc in range(2)]
            ones_bf = constp.tile([128, 1], BF)
            nc.vector.memset(ones_bf[:], 1.0)
            eps_c = constp.tile([128, 1], FP)
            nc.vector.memset(eps_c[:], EPS)
            onesrow_bf = constp.tile([1, 128], BF)
            nc.vector.memset(onesrow_bf[:], 1.0)

            # persistent
            qk_sb = [persp.tile([128, R], BF, name=f"qk{hc}") for hc in range(4)]
            alpha1 = persp.tile([128, 12], FP)
            beta1 = persp.tile([128, 12], FP)
            stats = persp.tile([128, 768], FP)
            statsg = persp.tile([128, 768], FP)
            gram_bf = persp.tile([128, 512], BF)
            meanx_bf = persp.tile([128, 2], BF)

            # W blocks for stats matmuls: (tiles, col0, width, flat-offset)
            wblocks = [(wqk_sb, 0, 512, 0),
                       (wv_sb, 0, 512, 512),
                       (wv_sb, 512, 512, 1024)]

            # ---------------- Phase A ----------------
            with tc.tile_pool(name="pA", bufs=2, space="PSUM") as pA, \
                 tc.tile_pool(name="sA", bufs=1) as sA:
                mean1s = sA.tile([128, 12], FP, tag="m1")
                vars_ = sA.tile([128, 12], FP, tag="vf")
                tmps = sA.tile([128, 12], FP, tag="tf")
                rstds = sA.tile([128, 12], FP, tag="rf")
                p_sb = [sA.tile([128, 1536], BF, tag=f"pp{c2}", name=f"p_sb{c2}")
                        for c2 in range(2)]
                with tc.tile_pool(name="xrp", bufs=1) as xrp, \
                     tc.tile_pool(name="pG", bufs=1, space="PSUM") as pG:
                    xr_sb = xrp.tile([128, 40 * C], BF)
                    for xh in (0, 1):
                        nc.sync.dma_start(
                            xr_sb[:, xh * 10 * C:(xh + 1) * 10 * C],
                            xr_d[:, xh * 10 * C:(xh + 1) * 10 * C])
                    for xh in (2, 3):
                        nc.scalar.dma_start(
                            xr_sb[:, xh * 10 * C:(xh + 1) * 10 * C],
                            xr_d[:, xh * 10 * C:(xh + 1) * 10 * C])
                    nc.vector.memset(stats[:, 512:768], 0.0)
                    for cc in range(2):
                        nc.sync.dma_start(xt_sb[cc][:],
                                          xt_d[cc * 128:(cc + 1) * 128, :])
                        nc.sync.dma_start(wqk_sb[cc][:],
                                          wqk_d[cc * 128:(cc + 1) * 128, :])
                        nc.sync.dma_start(wv_sb[cc][:],
                                          wv_d[cc * 128:(cc + 1) * 128, :])
                    nc.scalar.dma_start(wp_sb[:], wp_d[:])
                    nc.scalar.dma_start(eb_sb[:], eb_d[:])
                    nc.scalar.dma_start(g1c_sb[:], g1c_d[:])
                    nc.scalar.dma_start(b1c_sb[:], b1c_d[:])
                    nc.scalar.dma_start(id_sb[:], id_d[:])
                    gps = [pG.tile([128, C], FP, tag=f"g{cc}", name=f"gps{cc}")
                           for cc in range(2)]
                    csp = pG.tile([1, C], FP, tag="cs", name="csp")
                    # PE warm-up: ramp the tensor engine to full p-state while
                    # the xr DMA is in flight so the Gram runs at full clock.
                    wrm = sA.tile([128, 128], BF, tag="wrm")
                    nc.vector.memset(wrm[:], 0.0)
                    wps = pG.tile([128, 128], FP, tag="wp", name="wps")
                    for _ in range(20):
                        nc.tensor.matmul(wps[:], wrm[:], wrm[:])
                    rbord = ([rb for x in (0, 2) for rb in range(x * 10, x * 10 + 10)]
                             + [rb for x in (1, 3) for rb in range(x * 10, x * 10 + 10)])
                    for i, rb in enumerate(rbord):
                        fl = (i == 0)
                        ll = (i == 39)
                        nc.tensor.matmul(
                            gps[0][:],
                            xr_sb[:, rb * C: rb * C + 128],
                            xr_sb[:, rb * C:(rb + 1) * C],
                            start=fl, stop=ll)
                        nc.tensor.matmul(
                            gps[1][:, 128:256],
                            xr_sb[:, rb * C + 128: rb * C + 256],
                            xr_sb[:, rb * C + 128:(rb + 1) * C],
                            start=fl, stop=ll)
                        nc.tensor.matmul(
                            csp[:], ones_bf[:], xr_sb[:, rb * C:(rb + 1) * C],
                            start=fl, stop=ll)
                    nc.vector.tensor_copy(stats[:, 0:256], gps[0][:])
                    nc.vector.memset(stats[:, 256:384], 0.0)
                    nc.vector.tensor_copy(stats[:, 384:512], gps[1][:, 128:256])
                    nc.vector.tensor_copy(stats[0:1, 512:768], csp[:])
                bi = dramp.tile([128, 768], FP, tag="bi")
                bo = dramp.tile([128, 768], FP, tag="bo")
                nc.gpsimd.dma_start(bi[:], stats[:])
                nc.gpsimd.collective_compute(
                    "AllReduce", OP.add,
                    replica_groups=[list(range(NCORES))],
                    ins=[bi.opt()], outs=[bo.opt()])
                nc.gpsimd.dma_start(statsg[:], bo[:])

                # q/k matmuls (independent of stats; overlap the collective)
                for hc in range(4):
                    for rb in range(10):
                        pq = pA.tile([128, 512], FP, tag="pq")
                        for cc in range(2):
                            nc.tensor.matmul(
                                pq[:], wqk_sb[cc][:, hc * 128:(hc + 1) * 128],
                                xt_sb[cc][:, rb * 512:(rb + 1) * 512],
                                start=(cc == 0), stop=(cc == 1))
                        nc.scalar.copy(qk_sb[hc][:, rb * 512:(rb + 1) * 512],
                                       pq[:])

                # ---- BN1 stats math (gated on allreduce) ----
                with tc.tile_pool(name="pS1", bufs=1, space="PSUM") as pS1:
                    nc.gpsimd.tensor_copy(gram_bf[:, 0:128], statsg[:, 0:128])
                    nc.gpsimd.tensor_scalar(gram_bf[:, 128:256],
                                            statsg[:, 128:256], 2.0, None, OP.mult)
                    nc.gpsimd.tensor_copy(gram_bf[:, 384:512], statsg[:, 384:512])
                    for cc in range(2):
                        tpm = pS1.tile([128, 2], FP, tag="tp", bufs=1)
                        nc.tensor.transpose(
                            tpm[:, 0:1],
                            statsg[0:1, 512 + cc * 128:512 + (cc + 1) * 128],
                            id_sb[0:1, 0:1])
                        nc.vector.tensor_scalar(meanx_bf[:, cc:cc + 1], tpm[:, 0:1],
                                                1.0 / NT, None, OP.mult)
                    # mean1 [128,12] direct: out[:, hc] = sum_c W[c, hc-dims] mean_x[c]
                    m1p = pS1.tile([128, 12], FP, tag="m1p")
                    wcols = [(wqk_sb, 0), (wqk_sb, 128), (wqk_sb, 256), (wqk_sb, 384),
                             (wv_sb, 0), (wv_sb, 128), (wv_sb, 256), (wv_sb, 384),
                             (wv_sb, 512), (wv_sb, 640), (wv_sb, 768), (wv_sb, 896)]
                    for hc, (wt, c0) in enumerate(wcols):
                        for cc in range(2):
                            nc.tensor.matmul(m1p[:, hc:hc + 1],
                                             wt[cc][:, c0:c0 + 128],
                                             meanx_bf[:, cc:cc + 1],
                                             start=(cc == 0), stop=(cc == 1))
                    nc.vector.tensor_copy(mean1s[:], m1p[:])
                    # E2 decomposition using symmetric Gram:
                    # P0 = W0 .* (G00 W0); P1 = W1 .* (G11 W1 + 2 G01^T W0)
                    # where G01^T W0 is computed as M01[c2-chunk1, h] via
                    # stationary G01 [c1-part, c2-cols].
                    for (wt, c0, w, f0) in wblocks:
                        mm = pS1.tile([128, 512], FP, tag="mm", bufs=3)
                        nc.tensor.matmul(mm[:], gram_bf[:, 0:128],
                                         wt[0][:, c0:c0 + w])
                        nc.vector.tensor_tensor(p_sb[0][:, f0:f0 + w], mm[:],
                                                wt[0][:, c0:c0 + w], OP.mult)
                        mm2 = pS1.tile([128, 512], FP, tag="mm", bufs=3)
                        nc.tensor.matmul(mm2[:], gram_bf[:, 384:512],
                                         wt[1][:, c0:c0 + w], start=True, stop=False)
                        nc.tensor.matmul(mm2[:], gram_bf[:, 128:256],
                                         wt[0][:, c0:c0 + w], start=False, stop=True)
                        nc.vector.tensor_tensor(p_sb[1][:, f0:f0 + w], mm2[:],
                                                wt[1][:, c0:c0 + w], OP.mult)
                    # E2 [128,12] direct: out[:, hc] = sum_c2 P[c2, hc-dims]
                    e2p = pS1.tile([128, 12], FP, tag="e2p")
                    for hc in range(12):
                        for c2 in range(2):
                            nc.tensor.matmul(e2p[:, hc:hc + 1],
                                             p_sb[c2][:, hc * 128:(hc + 1) * 128],
                                             ones_bf[:],
                                             start=(c2 == 0), stop=(c2 == 1))
                    # var = E2/NT - mean1^2 + eps ; rstd = exp(-0.5 ln(var))
                    nc.vector.scalar_tensor_tensor(tmps[:], mean1s[:], -1.0,
                                                   mean1s[:], OP.mult, OP.mult)
                    nc.vector.scalar_tensor_tensor(vars_[:], e2p[:], 1.0 / NT,
                                                   tmps[:], OP.mult, OP.add)
                    nc.scalar.activation(tmps[:], vars_[:], AF.Ln, bias=eps_c[:])
                    nc.scalar.activation(rstds[:], tmps[:], AF.Exp, scale=-0.5)
                    nc.vector.tensor_tensor(alpha1[:], g1c_sb[:], rstds[:], OP.mult)
                    nc.vector.tensor_tensor(tmps[:], mean1s[:], alpha1[:], OP.mult)
                    nc.vector.tensor_tensor(beta1[:], b1c_sb[:], tmps[:], OP.subtract)

            if DBG_STOP == "A":
                for hc in range(4):
                    nc.vector.tensor_scalar(
                        qk_sb[hc][:], qk_sb[hc][:],
                        alpha1[:, hc:hc + 1], beta1[:, hc:hc + 1], OP.mult, OP.add)
                with tc.tile_pool(name="dbg", bufs=2) as dbgp:
                    for i, hc in enumerate((0, 2)):
                        for rb in range(10):
                            dq = dbgp.tile([128, 512], FP, tag="dq", name="dq")
                            nc.vector.tensor_copy(
                                dq[:], qk_sb[hc][:, rb * 512:(rb + 1) * 512])
                            nc.sync.dma_start(
                                y_d[i * 128:(i + 1) * 128, rb * 512:(rb + 1) * 512],
                                dq[:])
            else:
                # ---------- attention + output, pair-streamed ----------
                with tc.tile_pool(name="pSm", bufs=S_BUFS, space="PSUM") as pSm, \
                     tc.tile_pool(name="pAv", bufs=AV_BUFS, space="PSUM") as pAv, \
                     tc.tile_pool(name="pVm", bufs=VMM_BUFS, space="PSUM") as pVm, \
                     tc.tile_pool(name="pPj", bufs=PROJ_BUFS, space="PSUM") as pPj, \
                     tc.tile_pool(name="etp", bufs=ET_BUFS) as etp, \
                     tc.tile_pool(name="vtp", bufs=3) as vtp, \
                     tc.tile_pool(name="gpp", bufs=10) as gpp, \
                     tc.tile_pool(name="gpr", bufs=GP_BUFS) as gpr, \
                     tc.tile_pool(name="ysp", bufs=2) as ysp, \
                     tc.tile_pool(name="rrp", bufs=6) as rrp:
                    # v m-chunk windows in a pair: (name, row0-in-pair, width)
                    VCH = [("c0", 0, 128), ("c1", 128, 128),
                           ("c2", N, 128), ("c3", N + 128, 128)]
                    g_pairs = {}
                    for pair in range(NPAIR):
                        r0 = pair * PW
                        for hc in range(4):
                            nc.gpsimd.tensor_scalar(
                                qk_sb[hc][:, r0:r0 + PW], qk_sb[hc][:, r0:r0 + PW],
                                alpha1[:, hc:hc + 1], beta1[:, hc:hc + 1],
                                OP.mult, OP.add)
                        # v matmuls -> vT chunk tiles [m, 8*(128+1)] with alpha fold
                        vts = {}
                        for (cn, rr, wdt) in VCH:
                            vt = vtp.tile([128, NH * (DV + 1)], BF, tag=f"vt{cn}",
                                          name=f"vt{cn}")
                            vts[cn] = vt
                            vt3 = vt.rearrange("p (h x) -> p h x", x=DV + 1)
                            for dh in range(2):
                                vp = pVm.tile([128, 512], FP, tag="vp")
                                for cc in range(2):
                                    nc.tensor.matmul(
                                        vp[:], xt_sb[cc][:, r0 + rr:r0 + rr + wdt],
                                        wv_sb[cc][:, dh * 512:(dh + 1) * 512],
                                        start=(cc == 0), stop=(cc == 1))
                                nc.vector.tensor_copy(
                                    vt3[:, dh * 4:dh * 4 + 4, 0:128], vp[:])
                            nc.gpsimd.memset(vt[:, 128::129], 1.0)
                        vt = vtp.tile([128, NH * (DV + 1)], BF, tag="vt4", name="vt4")
                        vts["c4"] = vt
                        vt3 = vt.rearrange("p (h x) -> p h x", x=DV + 1)
                        for dh in range(2):
                            vp = pVm.tile([128, 512], FP, tag="vp")
                            for (bb, orow) in ((0, 0), (1, 64)):
                                for cc in range(2):
                                    nc.tensor.matmul(
                                        vp[orow:orow + 64, :],
                                        xt_sb[cc][:, r0 + bb * N + 256:
                                                 r0 + bb * N + 320],
                                        wv_sb[cc][:, dh * 512:(dh + 1) * 512],
                                        start=(cc == 0), stop=(cc == 1))
                            nc.vector.tensor_copy(
                                vt3[:, dh * 4:dh * 4 + 4, 0:128], vp[:])
                        nc.gpsimd.memset(vt[:, 128::129], 1.0)

                        g_pair = gpr.tile([128, NH * PW], BF, tag="gp", name="g_pair")
                        g_pairs[pair] = g_pair
                        for h in range(NH):
                            qc, qr = h // 4, 32 * (h % 4)
                            q0 = qk_sb[qc][qr:qr + 32, r0:r0 + N]
                            q1 = qk_sb[qc][qr:qr + 32, r0 + N:r0 + PW]
                            k0 = qk_sb[2 + qc][qr:qr + 32, r0:r0 + N]
                            k1 = qk_sb[2 + qc][qr:qr + 32, r0 + N:r0 + PW]
                            sdefs = [("s0", k0[:, 0:128], q0),
                                     ("s1", k0[:, 128:256], q0),
                                     ("s2", k1[:, 0:128], q1),
                                     ("s3", k1[:, 128:256], q1)]
                            ets = []
                            for (tg, kap, qap) in sdefs:
                                sp = pSm.tile([128, N], FP, tag="s", name="sp")
                                nc.tensor.matmul(sp[:], kap, qap,
                                                 tile_position=(qr, 0))
                                et = etp.tile([128, N], BF, tag=tg, name="et")
                                nc.scalar.activation(et[:], sp[:], AF.Exp, scale=SCALE)
                                ets.append(et)
                            sp = pSm.tile([128, N], FP, tag="s", name="sp4")
                            nc.tensor.matmul(sp[0:64, :], k0[:, 256:320], q0,
                                             tile_position=(qr, 0))
                            nc.tensor.matmul(sp[64:128, :], k1[:, 256:320], q1,
                                             tile_position=(qr, 64))
                            et4 = etp.tile([128, N], BF, tag="s4", name="et4")
                            nc.scalar.activation(et4[:], sp[:], AF.Exp, scale=SCALE)
                            ets.append(et4)
                            ebmap = [0, 1, 0, 1, 2]
                            for i, et in enumerate(ets):
                                ebs = eb_sb[:, (h * 3 + ebmap[i]) * N:
                                            (h * 3 + ebmap[i] + 1) * N]
                                if i < EBM_POOL:
                                    nc.gpsimd.tensor_tensor(et[:], et[:], ebs, OP.mult)
                                else:
                                    nc.vector.tensor_tensor(et[:], et[:], ebs, OP.mult)
                            avA = pAv.tile([128, 3 * 129], FP, tag="av", name="avA")
                            avB = pAv.tile([128, 3 * 129], FP, tag="av", name="avB")
                            regions = [(avA, 0, 0, 0), (avA, 129, 0, 1),
                                       (avB, 0, 1, 0), (avB, 129, 1, 1)]
                            vmap = [["c0", "c1", "c4"], ["c2", "c3", "c4"]]
                            emap = [[0, 1, 4], [2, 3, 4]]
                            for (dst, c0_, bb, ncx) in regions:
                                for mc in range(3):
                                    et = ets[emap[bb][mc]]
                                    vtt = vts[vmap[bb][mc]]
                                    if mc == 2:
                                        sr = bb * 64
                                        stp = et[sr:sr + 64, ncx * 128:ncx * 128 + 128]
                                        mvp = vtt[sr:sr + 64, h * 129:(h + 1) * 129]
                                    else:
                                        stp = et[:, ncx * 128:ncx * 128 + 128]
                                        mvp = vtt[:, h * 129:(h + 1) * 129]
                                    nc.tensor.matmul(dst[:, c0_:c0_ + 129], stp, mvp,
                                                     start=(mc == 0), stop=(mc == 2))
                            for bb in range(2):
                                orow = bb * 64
                                for mc in range(3):
                                    et = ets[emap[bb][mc]]
                                    vtt = vts[vmap[bb][mc]]
                                    if mc == 2:
                                        sr = bb * 64
                                        stp = et[sr:sr + 64, 256:320]
                                        mvp = vtt[sr:sr + 64, h * 129:(h + 1) * 129]
                                    else:
                                        stp = et[:, 256:320]
                                        mvp = vtt[:, h * 129:(h + 1) * 129]
                                    nc.tensor.matmul(
                                        avA[orow:orow + 64, 258:387], stp, mvp,
                                        start=(mc == 0), stop=(mc == 2))
                            rA = rrp.tile([128, 3], FP, tag="rA", name="rA")
                            rB = rrp.tile([128, 2], FP, tag="rB", name="rB")
                            nc.vector.reciprocal_approx_fast(rA[:], avA[:, 128::129])
                            nc.vector.reciprocal_approx_fast(rB[:],
                                                             avB[:, 128:300:129])
                            ddefs = [(avA, 0, rA, 0, 0), (avA, 129, rA, 1, 128),
                                     (avB, 0, rB, 0, N), (avB, 129, rB, 1, N + 128)]
                            for (src, c0_, rr_, ri, gc) in ddefs:
                                gp = gpp.tile([128, 128], BF, tag="gpre", name="gp")
                                nc.vector.tensor_scalar(
                                    gp[:], src[:, c0_:c0_ + 128], rr_[:, ri:ri + 1],
                                    None, OP.mult)
                                nc.sync.dma_start_transpose(
                                    g_pair[:, h * PW + gc:h * PW + gc + 128], gp[:])
                            gp = gpp.tile([128, 128], BF, tag="gpre", name="gp4")
                            nc.vector.tensor_scalar(
                                gp[:], avA[:, 258:386], rA[:, 2:3],
                                None, OP.mult)
                            nc.sync.dma_start_transpose(
                                g_pair[:, h * PW + 256:h * PW + 320], gp[0:64, :])
                            nc.sync.dma_start_transpose(
                                g_pair[:, h * PW + N + 256:h * PW + N + 320],
                                gp[64:128, :])
                        for h in range(NH):
                            nc.vector.tensor_scalar(
                                g_pair[:, h * PW:(h + 1) * PW],
                                g_pair[:, h * PW:(h + 1) * PW],
                                alpha1[:, 4 + h:5 + h], beta1[:, 4 + h:5 + h],
                                OP.mult, OP.add)
                        flush = ((pair % 2 == 1 and pair < 6) or pair >= 6)
                        if flush:
                            plist = [pair] if pair >= 6 else [pair - 1, pair]
                            for p2 in plist:
                                if pair >= 6:
                                    hw_ = NH * PW // 2
                                    for gh in range(2):
                                        nc.scalar.activation(
                                            g_pairs[p2][:, gh * hw_:(gh + 1) * hw_],
                                            g_pairs[p2][:, gh * hw_:(gh + 1) * hw_],
                                            AF.Gelu)
                                else:
                                    nc.scalar.activation(g_pairs[p2][:],
                                                         g_pairs[p2][:], AF.Gelu)
                            for p2 in plist:
                                gp2 = g_pairs.pop(p2)
                                rr0 = p2 * PW
                                for cc in range(2):
                                    yst = ysp.tile([128, PW], FP, tag=f"y{cc}",
                                                   name="yst")
                                    for half in range(2):
                                        if p2 == NPAIR - 1 and (cc + half) % 2 == 1:
                                            pp = pAv.tile([128, 3 * 129], FP,
                                                          tag="av", name="ppav")
                                            pp = pp[:, 0:N]
                                        else:
                                            pp = pPj.tile([128, N], FP, tag="pp")
                                        for h in range(NH):
                                            nc.tensor.matmul(
                                                pp[:],
                                                wp_sb[:, h * C + cc * 128:
                                                      h * C + cc * 128 + 128],
                                                gp2[:, h * PW + half * N:
                                                    h * PW + (half + 1) * N],
                                                start=(h == 0), stop=(h == NH - 1))
                                        nc.vector.tensor_copy(
                                            yst[:, half * N:(half + 1) * N], pp[:])
                                    if p2 == NPAIR - 1:
                                        for yh in range(2):
                                            nc.sync.dma_start(
                                                y_d[cc * 128:(cc + 1) * 128,
                                                    rr0 + yh * N:rr0 + (yh + 1) * N],
                                                yst[:, yh * N:(yh + 1) * N])
                                    else:
                                        nc.sync.dma_start(
                                            y_d[cc * 128:(cc + 1) * 128,
                                                rr0:rr0 + PW],
                                            yst[:])

    nc.compile()
    return nc


_PROG = None


def _get_prog():
    global _PROG
    if _PROG is None:
        _PROG = build_program()
    return _PROG


def _host_prep(x, Wqkv, g1, b1, ab, Wproj, g2, b2, idxs):
    x = np.asarray(x, dtype=np.float32)
    Wqkv = np.asarray(Wqkv, dtype=np.float32)
    Wproj = np.asarray(Wproj, dtype=np.float32)
    g1 = np.asarray(g1, np.float32)
    b1 = np.asarray(b1, np.float32)
    idxs = np.asarray(idxs)
    qrows = np.concatenate([np.arange(h * 192, h * 192 + 32) for h in range(NH)])
    krows = np.concatenate([np.arange(h * 192 + 32, h * 192 + 64) for h in range(NH)])
    vrows = np.concatenate([np.arange(h * 192 + 64, h * 192 + 192) for h in range(NH)])
    wqk = np.ascontiguousarray(Wqkv[np.concatenate([qrows, krows]), :].T).astype(
        ml_dtypes.bfloat16)                                     # (256, 512)
    wv = np.ascontiguousarray(Wqkv[vrows, :].T).astype(ml_dtypes.bfloat16)
    wp = np.ascontiguousarray(
        Wproj.reshape(C, NH, DV).transpose(2, 1, 0).reshape(DV, NH * C)).astype(
        ml_dtypes.bfloat16)                                     # (128, 2048)
    perm = np.concatenate([qrows, krows, vrows])
    g1c = np.ascontiguousarray(g1[perm].reshape(12, 128).T)
    b1c = np.ascontiguousarray(b1[perm].reshape(12, 128).T)
    ebm = np.exp(np.asarray(ab, np.float32))[:, idxs]           # (8, 320, 320)
    ebp = np.zeros((128, NH * 3 * N), np.float32)
    for h in range(NH):
        ebp[:, (h * 3) * N:(h * 3 + 1) * N] = ebm[h, 0:128, :]
        ebp[:, (h * 3 + 1) * N:(h * 3 + 2) * N] = ebm[h, 128:256, :]
        ebp[0:64, (h * 3 + 2) * N:(h * 3 + 3) * N] = ebm[h, 256:320, :]
        ebp[64:128, (h * 3 + 2) * N:(h * 3 + 3) * N] = ebm[h, 256:320, :]
    common = {
        "wqk": wqk, "wv": wv, "wp": wp,
        "eb": ebp.astype(ml_dtypes.bfloat16),
        "g1c": g1c, "b1c": b1c,
        "ident": np.eye(128, dtype=np.float32),
    }
    in_maps = []
    for c in range(NCORES):
        xc = np.ascontiguousarray(x[c * BL:(c + 1) * BL].reshape(R, C))
        m = dict(common)
        m["xr"] = np.ascontiguousarray(
            xc.reshape(40, 128, C).transpose(1, 0, 2).reshape(128, 40 * C)).astype(
            ml_dtypes.bfloat16)
        m["xt"] = np.ascontiguousarray(xc.T).astype(ml_dtypes.bfloat16)
        in_maps.append(m)
    return in_maps


def _postprocess(res, g2, b2):
    ys = [np.asarray(res.results[c]["y"], np.float32) for c in range(NCORES)]
    z = np.concatenate(
        [y.T.reshape(BL, N, C) for y in ys], axis=0)            # (128, 320, 256)
    zf = z.reshape(-1, C).astype(np.float64)
    m = zf.mean(axis=0)
    v = zf.var(axis=0)
    out = (z - m.astype(np.float32)) * (1.0 / np.sqrt(v + EPS)).astype(np.float32) \
        * np.asarray(g2, np.float32) + np.asarray(b2, np.float32)
    return out.astype(np.float32)


def _run(in_maps, g2, b2, trace=False):
    nc = _get_prog()
    res = run_bass_kernel_spmd(nc, in_maps, core_ids=list(range(NCORES)),
                               trace=trace)
    return _postprocess(res, g2, b2), res


def kernel(**inputs):
    out, _ = _run(_host_prep(**inputs), inputs["g2"], inputs["b2"])
    return out


def run_raw(**inputs):
    """Return raw per-core y tensors (for debugging)."""
    nc = _get_prog()
    res = run_bass_kernel_spmd(nc, _host_prep(**inputs),
                               core_ids=list(range(NCORES)))
    return [np.asarray(res.results[c]["y"], np.float32) for c in range(NCORES)]
